# revision 1
# baseline (speedup 1.0000x reference)
"""Trainium2 Bass kernel for nn_CNN_Comp_29240137351522 (dense_cnn).

Math:  y = |IFFT_N( FFT_N(x)^2 * C )[255:2303]|,  C = FFT_N(w0)^2 * FFT_N(wl) / N
with N = 2560 >= 2559 so the chained full convolutions (x*w0, autoconv, *wl)
are exact linear convolutions.

Device decomposition (per core, data-parallel over batch):
  N = N2*N1, N1=128, N2=20;  time n = n2*128+n1,  freq k = k1*20+k2
  F1 (contract n2, PE, block-diag over n1 i-blocks of 4, twiddle folded)
  F3 (contract n1, PE, shared W128 DFT)          -> X[k1, (k2,b)]
  square (ACT/DVE fused into F3 eviction)         -> Zr = Xr^2-Xi^2, P = Xr*Xi
  I1 (contract k1, PE, per-k2 weights G = C-row-scaled inverse DFT; the
      factor 2 of Zi=2P folded into G variants), bf16
  I2 (contract k2, PE, block-diag over n1 i-blocks of {6,6,4}, twiddle folded,
      output n2 in [1,18)), bf16
  |.| fused into I2 eviction; raw tiles stored to DRAM, unscrambled on host.

Host does data movement only: batch shard, column permutation of x (so PE
transposes produce the (i,n2)-partition layout directly), and the inverse
row->output-column unscramble of the raw result.
"""

import numpy as np
import ml_dtypes

import concourse.bass as bass
import concourse.bacc as bacc
import concourse.mybir as mybir
from concourse.tile import TileContext
from concourse.bass_utils import run_bass_kernel_spmd

# ---------------- static problem config ----------------
B, NX = 4096, 1024
K0, KL = 129, 257
N = 2560
N1, N2 = 128, 20
NCORES = 8
BCORE = B // NCORES          # 512
CHUNK = 256
NCHUNKS = BCORE // CHUNK     # 2
N2OUT = 17                   # n2 in [1,18)
CROP0 = 255
CLASS_NUM = 2048
IBLK_I2 = (6, 6, 4)
JOFS_I2 = (0, 6, 12)
YRAW_ROWS = 8 * sum(IBLK_I2) * N2OUT  # 2176

f32 = mybir.dt.float32
f32r = mybir.dt.float32r
bf16 = mybir.dt.bfloat16
AO = mybir.AluOpType
AF = mybir.ActivationFunctionType


def _w(num, den):
    return np.exp(-2j * np.pi * np.asarray(num, np.float64) / den)


# ---------------- host-side constant arrays ----------------
def _build_consts():
    c = {}
    n1g = np.arange(N1)
    k1g = np.arange(N1)
    k2g = np.arange(N2)
    n2g8 = np.arange(8)

    # F1 lhsT: [128, 640]; block (g,jj) at partitions [32jj,32jj+32), cols [80g,80g+80)
    # rows (il in 4)*8 + n2, cols il*20 + k2; value W20[n2,k2] * W2560^{n1 k2}, n1=16g+4jj+il
    f1 = np.zeros((128, 640), np.complex128)
    for g in range(8):
        for jj in range(4):
            for il in range(4):
                n1 = 16 * g + 4 * jj + il
                blk = _w(np.outer(n2g8, k2g), N2) * _w(n1 * k2g, N)[None, :]
                f1[32 * jj + il * 8 : 32 * jj + il * 8 + 8, 80 * g + il * 20 : 80 * g + (il + 1) * 20] = blk
    c["cf1r"] = f1.real.astype(np.float32)
    c["cf1i"] = f1.imag.astype(np.float32)
    c["cf1n"] = (-f1.imag).astype(np.float32)

    # F3 lhsT (shared): W128[n1,k1]
    w3 = _w(np.outer(n1g, k1g), N1)
    c["cw3r"] = w3.real.astype(np.float32)
    c["cw3i"] = w3.imag.astype(np.float32)
    c["cw3n"] = (-w3.imag).astype(np.float32)

    # I1 base: W128i[k1,n1] (fp32, G built on device)
    wi = _w(-np.outer(k1g, n1g), N1)
    c["cwir"] = wi.real.astype(np.float32)
    c["cwii"] = wi.imag.astype(np.float32)

    # I2 lhsT: [120, 2176]; per (g,j) cols [off,off+M_j); block-diag il:
    # rows il*20+k2, cols il*17+(n2-1); value W20^{-k2 n2} * W2560^{-n1 k2}
    n2out = np.arange(1, 18)
    i2 = np.zeros((120, 2176), np.complex128)
    off = 0
    for g in range(8):
        for j, cnt in enumerate(IBLK_I2):
            for il in range(cnt):
                n1 = 16 * g + JOFS_I2[j] + il
                blk = _w(-np.outer(k2g, n2out), N2) * _w(-n1 * k2g, N)[:, None]
                i2[il * 20 : (il + 1) * 20, off + il * 17 : off + (il + 1) * 17] = blk
            off += cnt * N2OUT
    c["ci2r"] = i2.real.astype(ml_dtypes.bfloat16)
    c["ci2i"] = i2.imag.astype(ml_dtypes.bfloat16)
    c["ci2n"] = (-i2.imag).astype(ml_dtypes.bfloat16)

    # weight-DFT rhs constants
    nh = np.arange(128)
    t129 = _w(np.outer(nh, k2g), N)
    c["ct1r"] = t129.real.astype(np.float32)
    c["ct1i"] = t129.imag.astype(np.float32)
    t257b = _w(np.outer(nh, k2g), N) * _w(k2g, 20)[None, :]
    c["ct2r"] = t257b.real.astype(np.float32)
    c["ct2i"] = t257b.imag.astype(np.float32)
    t129e = _w(k2g, 20)
    c["te1r"] = t129e.real.astype(np.float32).reshape(1, N2)
    c["te1i"] = t129e.imag.astype(np.float32).reshape(1, N2)
    t257e = _w(k2g, 10)
    c["te2r"] = t257e.real.astype(np.float32).reshape(1, N2)
    c["te2i"] = t257e.imag.astype(np.float32).reshape(1, N2)

    c["ones1"] = np.ones((1, 128), np.float32)
    c["ident"] = np.eye(128, dtype=np.float32)
    return c


CONSTS = _build_consts()


def host_x_perm():
    """perm[g*128 + i*8 + n2] = n2*128 + 16g + i"""
    perm = np.empty(NX, np.int64)
    for g in range(8):
        for i in range(16):
            for n2 in range(8):
                perm[g * 128 + i * 8 + n2] = n2 * 128 + 16 * g + i
    return perm


def yraw_maps():
    """row r of yraw -> output column (n-255), valid mask."""
    rows = []
    for g in range(8):
        for j, cnt in enumerate(IBLK_I2):
            for il in range(cnt):
                n1 = 16 * g + JOFS_I2[j] + il
                for q in range(N2OUT):
                    rows.append((q + 1) * 128 + n1)
    narr = np.array(rows)
    valid = (narr >= CROP0) & (narr < CROP0 + CLASS_NUM)
    return narr, valid


XPERM = host_x_perm()
YN, YVALID = yraw_maps()


# ---------------- bass kernel builder ----------------
def build_nc():
    nc = bacc.Bacc("TRN2", target_bir_lowering=False, debug=False, num_devices=NCORES)

    # DRAM tensors
    d = {}
    d["xp_r"] = nc.dram_tensor("xp_r", [BCORE, NX], f32, kind="ExternalInput")
    d["xp_i"] = nc.dram_tensor("xp_i", [BCORE, NX], f32, kind="ExternalInput")
    for nm, shape in [("w0r", [K0]), ("w0i", [K0]), ("wlr", [KL]), ("wli", [KL])]:
        d[nm] = nc.dram_tensor(nm, shape, f32, kind="ExternalInput")
    cdt = {"cf1r": f32r, "cf1i": f32r, "cf1n": f32r,
           "cw3r": f32r, "cw3i": f32r, "cw3n": f32r,
           "ci2r": bf16, "ci2i": bf16, "ci2n": bf16,
           "ones1": f32r}
    for nm, arr in CONSTS.items():
        d[nm] = nc.dram_tensor(nm, list(arr.shape), cdt.get(nm, f32), kind="ExternalInput")
    yraw = nc.dram_tensor("yraw", [YRAW_ROWS, BCORE], f32, kind="ExternalOutput")

    with TileContext(nc) as tc:
        with (
            tc.tile_pool(name="cp", bufs=1) as cp,         # consts + persistent
            tc.tile_pool(name="bp", bufs=1) as bp,         # big per-chunk tiles
            tc.tile_pool(name="sp", bufs=6) as sp,         # small rotating tiles
            tc.tile_pool(name="tp", bufs=3) as tp,         # f32 tmp tiles
            tc.tile_pool(name="psa", bufs=2, space="PSUM") as psa,   # 4 tags x 2 bufs = 8 banks
        ):
            # ---- load constants ----
            ct = {}
            big_consts = {"ci2r", "ci2i", "ci2n", "cwir", "cwii"}
            for nm, arr in CONSTS.items():
                t = cp.tile(list(arr.shape), cdt.get(nm, f32), tag=nm)
                eng = nc.gpsimd if nm in big_consts else nc.sync
                eng.dma_start(out=t[:], in_=d[nm][:, :] if arr.ndim == 2 else d[nm][:])
                ct[nm] = t

            # ---- load w0/wl pieces as [128,1] / [1,1] columns ----
            wc = {}
            for nm, src, lo, hi in [
                ("w0r_c", "w0r", 0, 128), ("w0i_c", "w0i", 0, 128),
                ("wlr_c1", "wlr", 0, 128), ("wli_c1", "wli", 0, 128),
                ("wlr_c2", "wlr", 128, 256), ("wli_c2", "wli", 128, 256),
            ]:
                t = cp.tile([128, 1], f32, tag=nm)
                nc.sync.dma_start(out=t[:], in_=d[src][lo:hi])
                wc[nm] = t
            for nm, src, pos in [("w0r_e", "w0r", 128), ("w0i_e", "w0i", 128),
                                 ("wlr_e", "wlr", 256), ("wli_e", "wli", 256)]:
                t = cp.tile([1, 1], f32, tag=nm)
                nc.sync.dma_start(out=t[:], in_=d[src][pos:pos + 1])
                wc[nm] = t

            # ---- weight DFT: W0, WL [128, 20] ----
            def build_rhs(tr, ti, cr_, ci_, out_r, out_i):
                # out_r = tr*cr - ti*ci ; out_i = ti*cr + tr*ci   (complex (tr+i ti)*(cr+i ci))
                tmp = tp.tile([tr.shape[0], N2], f32, tag="wtmp")
                nc.vector.tensor_scalar(tmp[:], ti[:], ci_[:], None, AO.mult)
                nc.vector.scalar_tensor_tensor(out_r[:], tr[:], cr_[:], tmp[:], AO.mult, AO.subtract)
                tmp2 = tp.tile([tr.shape[0], N2], f32, tag="wtmp2")
                nc.vector.tensor_scalar(tmp2[:], tr[:], ci_[:], None, AO.mult)
                nc.vector.scalar_tensor_tensor(out_i[:], ti[:], cr_[:], tmp2[:], AO.mult, AO.add)

            def weight_dft(chunks, tail, out_r, out_i):
                """chunks: list of (t_r_tile, t_i_tile, colr, coli); tail: (te_r, te_i, er, ei)."""
                ps_r = psa.tile([128, N2], f32, tag="pAr")
                ps_i = psa.tile([128, N2], f32, tag="pAi")
                rhs = []
                for (t_r, t_i, colr, coli) in chunks:
                    rr = sp.tile([128, N2], f32r, tag="wrhs_r")
                    ri = sp.tile([128, N2], f32r, tag="wrhs_i")
                    build_rhs(t_r, t_i, colr, coli, rr, ri)
                    rhs.append((rr, ri))
                te_r, te_i, er, ei = tail
                tr = sp.tile([1, N2], f32r, tag="wtail_r")
                ti_ = sp.tile([1, N2], f32r, tag="wtail_i")
                tmp = tp.tile([1, N2], f32, tag="wtmp3")
                nc.vector.tensor_scalar(tmp[:], te_i[:], ei[:], None, AO.mult)
                nc.vector.scalar_tensor_tensor(tr[:], te_r[:], er[:], tmp[:], AO.mult, AO.subtract)
                tmp2 = tp.tile([1, N2], f32, tag="wtmp4")
                nc.vector.tensor_scalar(tmp2[:], te_r[:], ei[:], None, AO.mult)
                nc.vector.scalar_tensor_tensor(ti_[:], te_i[:], er[:], tmp2[:], AO.mult, AO.add)
                # psum groups
                first = True
                for (rr, ri) in rhs:
                    nc.tensor.matmul(ps_r[:], ct["cw3r"][:], rr[:], start=first, stop=False)
                    nc.tensor.matmul(ps_r[:], ct["cw3n"][:], ri[:], start=False, stop=False)
                    first = False
                nc.tensor.matmul(ps_r[:], ct["ones1"][:1, :], tr[:], start=False, stop=True)
                first = True
                for (rr, ri) in rhs:
                    nc.tensor.matmul(ps_i[:], ct["cw3i"][:], rr[:], start=first, stop=False)
                    nc.tensor.matmul(ps_i[:], ct["cw3r"][:], ri[:], start=False, stop=False)
                    first = False
                nc.tensor.matmul(ps_i[:], ct["ones1"][:1, :], ti_[:], start=False, stop=True)
                nc.vector.tensor_copy(out_r[:], ps_r[:])
                nc.vector.tensor_copy(out_i[:], ps_i[:])

            W0r = cp.tile([128, N2], f32, tag="W0r")
            W0i = cp.tile([128, N2], f32, tag="W0i")
            weight_dft(
                [(ct["ct1r"], ct["ct1i"], wc["w0r_c"], wc["w0i_c"])],
                (ct["te1r"], ct["te1i"], wc["w0r_e"], wc["w0i_e"]),
                W0r, W0i,
            )
            WLr = cp.tile([128, N2], f32, tag="WLr")
            WLi = cp.tile([128, N2], f32, tag="WLi")
            weight_dft(
                [(ct["ct1r"], ct["ct1i"], wc["wlr_c1"], wc["wli_c1"]),
                 (ct["ct2r"], ct["ct2i"], wc["wlr_c2"], wc["wli_c2"])],
                (ct["te2r"], ct["te2i"], wc["wlr_e"], wc["wli_e"]),
                WLr, WLi,
            )

            # ---- C = W0^2 * WL / N  [128, 20] ----
            Cr = cp.tile([128, N2], f32, tag="Cr")
            Ci = cp.tile([128, N2], f32, tag="Ci")
            ta = tp.tile([128, N2], f32, tag="ca")
            tb = tp.tile([128, N2], f32, tag="cb")
            tm1 = tp.tile([128, N2], f32, tag="cm1")
            tm2 = tp.tile([128, N2], f32, tag="cm2")
            nc.vector.tensor_mul(tm1[:], W0r[:], W0r[:])
            nc.vector.tensor_mul(tm2[:], W0i[:], W0i[:])
            nc.vector.tensor_sub(ta[:], tm1[:], tm2[:])          # a = W0r^2 - W0i^2
            nc.vector.tensor_mul(tm1[:], W0r[:], W0i[:])
            nc.vector.tensor_add(tb[:], tm1[:], tm1[:])          # b = 2 W0r W0i
            nc.vector.tensor_mul(tm1[:], ta[:], WLr[:])
            nc.vector.tensor_mul(tm2[:], tb[:], WLi[:])
            nc.vector.tensor_sub(tm1[:], tm1[:], tm2[:])
            nc.scalar.mul(Cr[:], tm1[:], 1.0 / N)
            nc.vector.tensor_mul(tm1[:], ta[:], WLi[:])
            nc.vector.tensor_mul(tm2[:], tb[:], WLr[:])
            nc.vector.tensor_add(tm1[:], tm1[:], tm2[:])
            nc.scalar.mul(Ci[:], tm1[:], 1.0 / N)

            # ---- G variants (bf16): G_k2 = C[:,k2] row-scaled W128i ----
            Gr = cp.tile([128, N2 * 128], bf16, tag="Gr")
            Gi = cp.tile([128, N2 * 128], bf16, tag="Gi")
            Gn2 = cp.tile([128, N2 * 128], bf16, tag="Gn2")  # -2*Gi
            Gr2 = cp.tile([128, N2 * 128], bf16, tag="Gr2")  # 2*Gr
            for k2 in range(N2):
                cr_ = Cr[:, k2 : k2 + 1]
                ci_ = Ci[:, k2 : k2 + 1]
                sl = slice(k2 * 128, (k2 + 1) * 128)
                gt = tp.tile([128, 128], f32, tag="gtmp")
                nc.vector.tensor_scalar(gt[:], ct["cwii"][:], ci_, None, AO.mult)
                nc.vector.scalar_tensor_tensor(Gr[:, sl], ct["cwir"][:], cr_, gt[:], AO.mult, AO.subtract)
                gt2 = tp.tile([128, 128], f32, tag="gtmp2")
                nc.vector.tensor_scalar(gt2[:], ct["cwir"][:], ci_, None, AO.mult)
                nc.vector.scalar_tensor_tensor(Gi[:, sl], ct["cwii"][:], cr_, gt2[:], AO.mult, AO.add)
                nc.scalar.mul(Gn2[:, sl], Gi[:, sl], -2.0)
                nc.scalar.mul(Gr2[:, sl], Gr[:, sl], 2.0)

            # ---- per-chunk pipeline ----
            i2_offs = []
            off = 0
            for g in range(8):
                for j, cnt in enumerate(IBLK_I2):
                    i2_offs.append((g, j, cnt, off))
                    off += cnt * N2OUT

            for c in range(NCHUNKS):
                # T-in: load + transpose
                xn_r = bp.tile([128, 2048], f32, tag="big1")
                xn_i = bp.tile([128, 2048], f32, tag="big2")
                for h in range(2):
                    rows = slice(c * CHUNK + h * 128, c * CHUNK + (h + 1) * 128)
                    nc.sync.dma_start(out=xn_r[:, h * 1024 : (h + 1) * 1024], in_=d["xp_r"][rows, :])
                    nc.sync.dma_start(out=xn_i[:, h * 1024 : (h + 1) * 1024], in_=d["xp_i"][rows, :])
                xt_r = bp.tile([128, 2048], f32r, tag="big3")
                xt_i = bp.tile([128, 2048], f32r, tag="big4")
                for plane, xn, xt in [(0, xn_r, xt_r), (1, xn_i, xt_i)]:
                    for h in range(2):
                        for g in range(8):
                            tps = psa.tile([128, 512], f32, tag="pBr")
                            nc.tensor.transpose(
                                tps[:128, :128],
                                xn[:, h * 1024 + g * 128 : h * 1024 + (g + 1) * 128],
                                ct["ident"][:],
                            )
                            nc.scalar.activation(
                                xt[:, g * 256 + h * 128 : g * 256 + (h + 1) * 128],
                                tps[:128, :128], AF.Copy,
                            )

                # F1 + pivot-C into plane-interleaved Abig [n1, k2*512 + plane*256 + b]
                Abig = bp.tile([128, 10240], f32r, tag="Abig")
                for g in range(8):
                    for jj in range(4):
                        pw = slice(32 * jj, 32 * jj + 32)
                        cwd = slice(80 * g, 80 * (g + 1))
                        rr = xt_r[pw, g * 256 : (g + 1) * 256]
                        ri = xt_i[pw, g * 256 : (g + 1) * 256]
                        lr = ct["cf1r"][pw, cwd]
                        li = ct["cf1i"][pw, cwd]
                        ln = ct["cf1n"][pw, cwd]
                        tpos = (32 * jj, 0)
                        pr = psa.tile([80, 256], f32, tag="pAr")
                        pi = psa.tile([80, 256], f32, tag="pAi")
                        nc.tensor.matmul(pr[:], lr, rr, start=True, stop=False, tile_position=tpos)
                        nc.tensor.matmul(pr[:], ln, ri, start=False, stop=True, tile_position=tpos)
                        nc.tensor.matmul(pi[:], li, rr, start=True, stop=False, tile_position=tpos)
                        nc.tensor.matmul(pi[:], lr, ri, start=False, stop=True, tile_position=tpos)
                        ag = sp.tile([80, 512], f32r, tag="ag")
                        nc.scalar.activation(ag[:, 0:256], pr[:], AF.Copy)
                        nc.vector.tensor_copy(ag[:, 256:512], pi[:])
                        # pivot: [(il,k2), (plane,b)] -> Abig[n1, k2*512+plane*256+b]
                        nc.sync.dma_start(
                            out=bass.AP(Abig.tensor, Abig[:].offset + (16 * g + 4 * jj) * 10240,
                                        [[10240, 4], [1, 10240]]),
                            in_=ag[:],
                        )

                # F3 + fused square eviction
                Zr = bp.tile([128, 5120], bf16, tag="Zr")
                Pt = bp.tile([128, 5120], bf16, tag="Pt")
                for k2 in range(N2):
                    asl_r = slice(k2 * 512, k2 * 512 + 256)
                    asl_i = slice(k2 * 512 + 256, k2 * 512 + 512)
                    zsl = slice(k2 * 256, (k2 + 1) * 256)
                    pr = psa.tile([128, 256], f32, tag="pBr")
                    pi = psa.tile([128, 256], f32, tag="pBi")
                    nc.tensor.matmul(pr[:], ct["cw3r"][:], Abig[:, asl_r], start=True, stop=False)
                    nc.tensor.matmul(pr[:], ct["cw3n"][:], Abig[:, asl_i], start=False, stop=True)
                    nc.tensor.matmul(pi[:], ct["cw3i"][:], Abig[:, asl_r], start=True, stop=False)
                    nc.tensor.matmul(pi[:], ct["cw3r"][:], Abig[:, asl_i], start=False, stop=True)
                    m1 = tp.tile([128, 256], f32, tag="sq1")
                    m2 = tp.tile([128, 256], f32, tag="sq2")
                    xi_s = tp.tile([128, 256], f32, tag="xis")
                    nc.vector.tensor_copy(xi_s[:], pi[:])
                    nc.scalar.activation(m1[:], pr[:], AF.Square)
                    nc.scalar.activation(m2[:], pi[:], AF.Square)
                    nc.vector.tensor_sub(Zr[:, zsl], m1[:], m2[:])
                    nc.vector.tensor_mul(Pt[:, zsl], pr[:], xi_s[:])

                # I1 (bf16); evict into plane-interleaved Ubig
                Ubig = bp.tile([128, 10240], bf16, tag="big1")
                for k2 in range(N2):
                    zsl = slice(k2 * 256, (k2 + 1) * 256)
                    gsl = slice(k2 * 128, (k2 + 1) * 128)
                    pr = psa.tile([128, 256], f32, tag="pAr")
                    pi = psa.tile([128, 256], f32, tag="pAi")
                    nc.tensor.matmul(pr[:], Gr[:, gsl], Zr[:, zsl], start=True, stop=False)
                    nc.tensor.matmul(pr[:], Gn2[:, gsl], Pt[:, zsl], start=False, stop=True)
                    nc.tensor.matmul(pi[:], Gi[:, gsl], Zr[:, zsl], start=True, stop=False)
                    nc.tensor.matmul(pi[:], Gr2[:, gsl], Pt[:, zsl], start=False, stop=True)
                    nc.scalar.activation(Ubig[:, k2 * 512 : k2 * 512 + 256], pr[:], AF.Copy)
                    nc.vector.tensor_copy(Ubig[:, k2 * 512 + 256 : (k2 + 1) * 512], pi[:])

                # pivot-D: one DMA per (g,j) into interleaved u2 [(il,k2), idx*512+plane*256+b]
                u2 = bp.tile([120, 24 * 512], bf16, tag="big2")
                for idx, (g, j, cnt, off) in enumerate(i2_offs):
                    n1_0 = 16 * g + JOFS_I2[j]
                    nc.sync.dma_start(
                        out=bass.AP(u2.tensor, u2[:].offset + idx * 512,
                                    [[24 * 512, cnt * 20], [1, 512]]),
                        in_=bass.AP(Ubig.tensor, Ubig[:].offset + n1_0 * 10240,
                                    [[10240, cnt], [1, 10240]]),
                    )

                # I2 (bf16) + fused abs + store
                for idx, (g, j, cnt, off) in enumerate(i2_offs):
                    Kj, Mj = cnt * 20, cnt * N2OUT
                    csl = slice(off, off + Mj)
                    usl_r = slice(idx * 512, idx * 512 + 256)
                    usl_i = slice(idx * 512 + 256, (idx + 1) * 512)
                    pr = psa.tile([102, 256], f32, tag="pBr")
                    pi = psa.tile([102, 256], f32, tag="pBi")
                    nc.tensor.matmul(pr[:Mj, :], ct["ci2r"][:Kj, csl], u2[:Kj, usl_r], start=True, stop=False)
                    nc.tensor.matmul(pr[:Mj, :], ct["ci2n"][:Kj, csl], u2[:Kj, usl_i], start=False, stop=True)
                    nc.tensor.matmul(pi[:Mj, :], ct["ci2i"][:Kj, csl], u2[:Kj, usl_r], start=True, stop=False)
                    nc.tensor.matmul(pi[:Mj, :], ct["ci2r"][:Kj, csl], u2[:Kj, usl_i], start=False, stop=True)
                    s1 = tp.tile([102, 256], f32, tag="ab1")
                    s2 = tp.tile([102, 256], f32, tag="ab2")
                    nc.scalar.activation(s1[:Mj, :], pr[:Mj, :], AF.Square)
                    nc.scalar.activation(s2[:Mj, :], pi[:Mj, :], AF.Square)
                    nc.vector.tensor_add(s1[:Mj, :], s1[:Mj, :], s2[:Mj, :])
                    ya = sp.tile([102, 256], f32, tag="yab")
                    nc.scalar.activation(ya[:Mj, :], s1[:Mj, :], AF.Sqrt)
                    nc.gpsimd.dma_start(
                        out=yraw[off : off + Mj, c * CHUNK : (c + 1) * CHUNK],
                        in_=ya[:Mj, :],
                    )

    nc.compile()
    return nc


_NC_CACHE = None
_LAST_IN_MAPS = None


def kernel(**inputs):
    global _NC_CACHE
    x_real = np.ascontiguousarray(inputs["x_real"], dtype=np.float32)
    x_imag = np.ascontiguousarray(inputs["x_imag"], dtype=np.float32)
    w0_real = np.ascontiguousarray(inputs["w0_real"], dtype=np.float32)
    w0_imag = np.ascontiguousarray(inputs["w0_imag"], dtype=np.float32)
    wl_real = np.ascontiguousarray(inputs["wl_real"], dtype=np.float32)
    wl_imag = np.ascontiguousarray(inputs["wl_imag"], dtype=np.float32)

    xp_r = x_real[:, XPERM]
    xp_i = x_imag[:, XPERM]

    const_maps = {}
    for nm, arr in CONSTS.items():
        const_maps[nm] = np.ascontiguousarray(arr)
    in_maps = []
    for cid in range(NCORES):
        rows = slice(cid * BCORE, (cid + 1) * BCORE)
        m = {
            "xp_r": np.ascontiguousarray(xp_r[rows]),
            "xp_i": np.ascontiguousarray(xp_i[rows]),
            "w0r": w0_real, "w0i": w0_imag,
            "wlr": wl_real, "wli": wl_imag,
        }
        m.update(const_maps)
        in_maps.append(m)

    global _LAST_IN_MAPS
    _LAST_IN_MAPS = in_maps
    if _NC_CACHE is None:
        _NC_CACHE = build_nc()
    res = run_bass_kernel_spmd(_NC_CACHE, in_maps, core_ids=list(range(NCORES)))

    out = np.empty((B, CLASS_NUM), np.float32)
    cols = YN[YVALID] - CROP0
    for cid in range(NCORES):
        yraw = res.results[cid]["yraw"]  # [2176, 512]
        out[cid * BCORE : (cid + 1) * BCORE, cols] = yraw[YVALID].T
    return out



# revision 23
# speedup vs baseline: 1.5945x; 1.5945x over previous
"""Trainium2 Bass kernel for nn_CNN_Comp_29240137351522 (dense_cnn).

Math:  y = |IFFT_N( FFT_N(x)^2 * C )[255:2303]|,  C = FFT_N(w0)^2 * FFT_N(wl) / N
with N = 2304 = 128*18.  2304 >= 2303-ish: circular aliasing only contaminates
samples n < 255, which the center crop [255, 2303) never reads, so the chained
full convolutions are exact on the cropped output.

Device decomposition (per core, data-parallel over batch, b = 512 samples):
  N = N1*N2, N1=128, N2=18;  time n = n2*128+n1,  freq k = k1*18+k2
  F1 (contract n2<8, PE, 19 blocks of <=7 n1-values, twiddle folded)
  F3 (contract n1, PE, shared W128 DFT, bf16)      -> X[k1, (k2,b)]
  square (ACT/DVE/Pool fused eviction)             -> Zr4 = 4(Xr^2-Xi^2), Zi2 = 4XrXi
  I1 (contract k1, PE, per-k2 G = (C/4N)-row-scaled inverse DFT), bf16
  I2 (contract k2, PE, 19 blocks of <=7 n1'-values, twiddle folded,
      exactly 16 valid n2' outputs per n1' -> 2048 rows), bf16
  |.| fused into I2 eviction; raw bf16 tiles stored to DRAM, unscrambled on host.

Host does data movement only: batch shard, pre-transposed/permuted copy of x
(so no on-device transposes are needed), and the inverse row->output-column
unscramble of the raw result.
"""

import numpy as np
import ml_dtypes

import concourse.bass as bass
import concourse.bacc as bacc
import concourse.mybir as mybir
from concourse.tile import TileContext
from concourse.bass_utils import run_bass_kernel_spmd

# ---------------- static problem config ----------------
B, NX = 4096, 1024
K0, KL = 129, 257
N = 2304
N1, N2 = 128, 18
NCORES = 8
BCORE = B // NCORES          # 512
FCNT = [7] * 18 + [2]        # il-count per block; n1 = 7*b + il
NB = 19
CROP0 = 255
CLASS_NUM = 2048
XROWS = 1280                 # 10 tiles of 128 rows, 2 blocks per tile
PITCH_A = N2 * 1024          # Abig free size  (k2, plane, b)
PITCH_U = N2 * 1024          # Ubig free size  (k2, plane, b)

f32 = mybir.dt.float32
f32r = mybir.dt.float32r
bf16 = mybir.dt.bfloat16
AO = mybir.AluOpType
AF = mybir.ActivationFunctionType


def _w(num, den):
    return np.exp(-2j * np.pi * np.asarray(num, np.float64) / den)


# ---------------- host-side constant arrays ----------------
def _build_consts():
    c = {}
    k1g = np.arange(N1)
    k2g = np.arange(N2)
    nh = np.arange(128)

    # F1 lhsT: [112, 10*126]; pair slot s=b%2 at rows [56s,56s+56);
    # block b at cols [(b//2)*126, ...); rows (il)*8+n2, cols il*18+k2;
    # value W18[n2,k2] * W2304^{n1 k2}, n1 = 7b+il
    f1 = np.zeros((120, 10 * 126), np.complex128)
    n2g8 = np.arange(8)
    for b in range(NB):
        s, t = b % 2, b // 2
        for il in range(FCNT[b]):
            n1 = 7 * b + il
            blk = _w(np.outer(n2g8, k2g), N2) * _w(n1 * k2g, N)[None, :]
            f1[64 * s + il * 8: 64 * s + il * 8 + 8,
               t * 126 + il * 18: t * 126 + (il + 1) * 18] = blk
    c["cf1r"] = f1.real.astype(np.float32)
    c["cf1i"] = f1.imag.astype(np.float32)
    c["cf1n"] = (-f1.imag).astype(np.float32)

    # F3 lhsT (shared, bf16): W128[n1,k1]
    w3 = _w(np.outer(nh, k1g), N1)
    c["cw3r"] = w3.real.astype(ml_dtypes.bfloat16)
    c["cw3i"] = w3.imag.astype(ml_dtypes.bfloat16)
    c["cw3n"] = (-w3.imag).astype(ml_dtypes.bfloat16)
    # f32 copies for the weight DFT (precision)
    c["cw3rf"] = w3.real.astype(np.float32)
    c["cw3if"] = w3.imag.astype(np.float32)
    c["cw3nf"] = (-w3.imag).astype(np.float32)

    # I1 base: W128i[k1,n1] (f32, G built on device)
    wi = _w(-np.outer(k1g, np.arange(N1)), N1)
    c["cwir"] = wi.real.astype(np.float32)
    c["cwii"] = wi.imag.astype(np.float32)

    # I2 lhsT: [126, 2048]; block b cols [b*112, b*112+16*cnt);
    # rows il*18+k2, cols il*16+j; n2'(j) = (1 if n1'==127 else 2)+j
    i2 = np.zeros((126, 2048), np.complex128)
    for b in range(NB):
        for il in range(FCNT[b]):
            n1p = 7 * b + il
            base = 1 if n1p == 127 else 2
            n2p = base + np.arange(16)
            blk = _w(-np.outer(k2g, n2p), N2) * _w(-n1p * k2g, N)[:, None]
            i2[il * 18: (il + 1) * 18,
               b * 112 + il * 16: b * 112 + (il + 1) * 16] = blk
    c["ci2r"] = i2.real.astype(ml_dtypes.bfloat16)
    c["ci2i"] = i2.imag.astype(ml_dtypes.bfloat16)
    c["ci2n"] = (-i2.imag).astype(ml_dtypes.bfloat16)

    # weight-DFT rhs constants
    t129 = _w(np.outer(nh, k2g), N)
    c["ct1r"] = t129.real.astype(np.float32)
    c["ct1i"] = t129.imag.astype(np.float32)
    te1 = _w(k2g, N2)
    t257b = t129 * te1[None, :]
    c["ct2r"] = t257b.real.astype(np.float32)
    c["ct2i"] = t257b.imag.astype(np.float32)
    c["te1r"] = te1.real.astype(np.float32).reshape(1, N2)
    c["te1i"] = te1.imag.astype(np.float32).reshape(1, N2)
    te2 = _w(k2g, 9)
    c["te2r"] = te2.real.astype(np.float32).reshape(1, N2)
    c["te2i"] = te2.imag.astype(np.float32).reshape(1, N2)

    c["ones1"] = np.ones((1, 128), np.float32)

    # ---- pack into few DRAM tensors (fewer DMAs; device slices views) ----
    p = {}
    p["wdftf"] = np.concatenate([c[k] for k in
        ["ct1r", "ct1i", "ct2r", "ct2i", "cw3rf", "cw3if", "cw3nf"]], axis=1)
    p["smalls"] = np.concatenate([c[k] for k in
        ["te1r", "te1i", "te2r", "te2i", "ones1"]], axis=1)
    p["cwaux"] = np.concatenate([c["cwir"], c["cwii"]], axis=1)
    p["cf1p"] = np.concatenate([c["cf1r"], c["cf1i"], c["cf1n"]], axis=1)
    p["cw3p"] = np.concatenate([c["cw3r"], c["cw3i"], c["cw3n"]], axis=1)
    p["ci2p"] = np.concatenate([c["ci2r"], c["ci2i"], c["ci2n"]], axis=1)
    return p


CONSTS = _build_consts()


def _xrow_map():
    """DRAM row r of xp_t -> original column n of x (or -1 for pad)."""
    rmap = np.full(XROWS, -1, np.int64)
    for r in range(XROWS):
        t, rr = divmod(r, 128)
        s, rrr = divmod(rr, 64)
        if rrr >= 56:
            continue
        il, n2 = divmod(rrr, 8)
        b = 2 * t + s
        if b >= NB or il >= FCNT[b]:
            continue
        n1 = 7 * b + il
        rmap[r] = n2 * 128 + n1
    return rmap


def _ycol_map():
    """yraw row r -> output column (n - 255)."""
    cols = np.empty(2048, np.int64)
    r = 0
    for b in range(NB):
        for il in range(FCNT[b]):
            n1p = 7 * b + il
            base = 1 if n1p == 127 else 2
            for j in range(16):
                cols[b * 112 + il * 16 + j] = (base + j) * 128 + n1p - CROP0
                r += 1
    return cols


XRMAP = _xrow_map()
YCOLS = _ycol_map()
YINV = np.empty(2048, np.int64)
YINV[YCOLS] = np.arange(2048)


# ---------------- bass kernel builder ----------------
DEBUG_DUMP = False


def build_nc():
    nc = bacc.Bacc("TRN2", target_bir_lowering=False, debug=False, num_devices=NCORES)

    d = {}
    d["xtr"] = nc.dram_tensor("xtr", [XROWS, BCORE], f32r, kind="ExternalInput")
    d["xti"] = nc.dram_tensor("xti", [XROWS, BCORE], f32r, kind="ExternalInput")
    d["wpack"] = nc.dram_tensor("wpack", [128, 6], f32, kind="ExternalInput")
    d["wtail"] = nc.dram_tensor("wtail", [1, 4], f32, kind="ExternalInput")
    cdt = {"cf1p": f32r, "cw3p": bf16, "ci2p": bf16}
    for nm, arr in CONSTS.items():
        d[nm] = nc.dram_tensor(nm, list(arr.shape), cdt.get(nm, f32), kind="ExternalInput")
    yraw = nc.dram_tensor("yraw", [2048, BCORE], bf16, kind="ExternalOutput")
    dbg = {}
    if DEBUG_DUMP:
        dbg["dAbig"] = nc.dram_tensor("dAbig", [128, N2 * 1024], bf16, kind="ExternalOutput")
        dbg["dZr"] = nc.dram_tensor("dZr", [128, N2 * 512], bf16, kind="ExternalOutput")
        dbg["dZi"] = nc.dram_tensor("dZi", [128, N2 * 512], bf16, kind="ExternalOutput")
        dbg["dU"] = nc.dram_tensor("dU", [128, N2 * 1024], bf16, kind="ExternalOutput")
        dbg["dxt"] = nc.dram_tensor("dxt", [128, 10 * BCORE], f32r, kind="ExternalOutput")

    with TileContext(nc) as tc:
        with (
            tc.tile_pool(name="cp", bufs=1) as cp,          # consts + persistent
            tc.tile_pool(name="bp", bufs=1) as bp,          # big tiles (tag-aliased)
            tc.tile_pool(name="sp", bufs=2) as sp,          # rotating bf16 tiles
            tc.tile_pool(name="tp", bufs=2) as tp,          # rotating f32 tmps
            tc.tile_pool(name="psa", bufs=2, space="PSUM") as psa,  # 2 tags x 2 bufs x 2 banks
        ):
            # ---- load packed weights/consts (few DMAs; critical-path first) ----
            wpk = cp.tile([128, 6], f32, tag="wpk", name="wpk")
            nc.sync.dma_start(out=wpk[:], in_=d["wpack"][:, :])
            wtl = cp.tile([1, 4], f32, tag="wtl", name="wtl")
            nc.sync.dma_start(out=wtl[:], in_=d["wtail"][:, :])
            wc = {"w0r_c": wpk[:, 0:1], "w0i_c": wpk[:, 1:2],
                  "wlr_c1": wpk[:, 2:3], "wli_c1": wpk[:, 3:4],
                  "wlr_c2": wpk[:, 4:5], "wli_c2": wpk[:, 5:6],
                  "w0r_e": wtl[:, 0:1], "w0i_e": wtl[:, 1:2],
                  "wlr_e": wtl[:, 2:3], "wli_e": wtl[:, 3:4]}

            pk = {}
            for nm, eng in [("wdftf", nc.sync), ("smalls", nc.sync), ("cf1p", nc.sync),
                            ("cwaux", nc.scalar), ("cw3p", nc.scalar), ("ci2p", nc.scalar)]:
                arr = CONSTS[nm]
                t = cp.tile(list(arr.shape), cdt.get(nm, f32), tag=nm, name=nm)
                eng.dma_start(out=t[:], in_=d[nm][:, :])
                pk[nm] = t
            ct = {
                "ct1r": pk["wdftf"][:, 0:18], "ct1i": pk["wdftf"][:, 18:36],
                "ct2r": pk["wdftf"][:, 36:54], "ct2i": pk["wdftf"][:, 54:72],
                "cw3rf": pk["wdftf"][:, 72:200], "cw3if": pk["wdftf"][:, 200:328],
                "cw3nf": pk["wdftf"][:, 328:456],
                "te1r": pk["smalls"][:1, 0:18], "te1i": pk["smalls"][:1, 18:36],
                "te2r": pk["smalls"][:1, 36:54], "te2i": pk["smalls"][:1, 54:72],
                "ones1": pk["smalls"][:1, 72:200],
                "cwir": pk["cwaux"][:, 0:128], "cwii": pk["cwaux"][:, 128:256],
                "cf1r": pk["cf1p"][:, 0:1260], "cf1i": pk["cf1p"][:, 1260:2520],
                "cf1n": pk["cf1p"][:, 2520:3780],
                "cw3r": pk["cw3p"][:, 0:128], "cw3i": pk["cw3p"][:, 128:256],
                "cw3n": pk["cw3p"][:, 256:384],
                "ci2r": pk["ci2p"][:, 0:2048], "ci2i": pk["ci2p"][:, 2048:4096],
                "ci2n": pk["ci2p"][:, 4096:6144],
            }

            # ---- load x planes in 4 chunks per plane (F1 starts early) ----
            xt_r = bp.tile([128, 10 * BCORE], f32r, tag="big1", name="xt_r")
            xt_i = bp.tile([128, 10 * BCORE], f32r, tag="big2", name="xt_i")
            for c0, ntile in ((0, 3), (3, 3), (6, 3), (9, 1)):
                for xt, srcn in ((xt_r, "xtr"), (xt_i, "xti")):
                    nc.sync.dma_start(
                        out=xt[:, 512 * c0: 512 * (c0 + ntile)],
                        in_=bass.AP(d[srcn][:, :].tensor, 128 * c0 * BCORE,
                                    [[BCORE, 128], [128 * BCORE, ntile], [1, BCORE]]),
                    )

            # ---- weight DFT: W0, WL [128, 18] ----
            def build_rhs(tr, ti, cr_, ci_, out_r, out_i):
                tmp = tp.tile([tr.shape[0], N2], f32, tag="wtmp", name="wtmp")
                nc.vector.tensor_scalar(tmp[:], ti, ci_, None, AO.mult)
                nc.vector.scalar_tensor_tensor(out_r[:], tr, cr_, tmp[:], AO.mult, AO.subtract)
                tmp2 = tp.tile([tr.shape[0], N2], f32, tag="wtmp2", name="wtmp2")
                nc.vector.tensor_scalar(tmp2[:], tr, ci_, None, AO.mult)
                nc.vector.scalar_tensor_tensor(out_i[:], ti, cr_, tmp2[:], AO.mult, AO.add)

            def weight_dft(chunks, tail, out_r, out_i):
                ps_r = psa.tile([128, 1024], f32, tag="pA", name="ps_r")
                ps_i = psa.tile([128, 1024], f32, tag="pB", name="ps_i")
                rhs = []
                for (t_r, t_i, colr, coli) in chunks:
                    rr = sp.tile([128, N2], f32, tag="wrhs_r", name="wrhs_r")
                    ri = sp.tile([128, N2], f32, tag="wrhs_i", name="wrhs_i")
                    build_rhs(t_r, t_i, colr, coli, rr, ri)
                    rhs.append((rr, ri))
                te_r, te_i, er, ei = tail
                tr_ = sp.tile([1, N2], f32, tag="wtail_r", name="wtail_r")
                ti_ = sp.tile([1, N2], f32, tag="wtail_i", name="wtail_i")
                tmp = tp.tile([1, N2], f32, tag="wtmp3", name="wtmp3")
                nc.vector.tensor_scalar(tmp[:], te_i, ei, None, AO.mult)
                nc.vector.scalar_tensor_tensor(tr_[:], te_r, er, tmp[:], AO.mult, AO.subtract)
                tmp2 = tp.tile([1, N2], f32, tag="wtmp4", name="wtmp4")
                nc.vector.tensor_scalar(tmp2[:], te_r, ei, None, AO.mult)
                nc.vector.scalar_tensor_tensor(ti_[:], te_i, er, tmp2[:], AO.mult, AO.add)
                first = True
                for (rr, ri) in rhs:
                    nc.tensor.matmul(ps_r[:, :N2], ct["cw3rf"], rr[:], start=first, stop=False)
                    nc.tensor.matmul(ps_r[:, :N2], ct["cw3nf"], ri[:], start=False, stop=False)
                    first = False
                nc.tensor.matmul(ps_r[:, :N2], ct["ones1"], tr_[:], start=False, stop=True)
                first = True
                for (rr, ri) in rhs:
                    nc.tensor.matmul(ps_i[:, :N2], ct["cw3if"], rr[:], start=first, stop=False)
                    nc.tensor.matmul(ps_i[:, :N2], ct["cw3rf"], ri[:], start=False, stop=False)
                    first = False
                nc.tensor.matmul(ps_i[:, :N2], ct["ones1"], ti_[:], start=False, stop=True)
                nc.vector.tensor_copy(out_r[:], ps_r[:, :N2])
                nc.vector.tensor_copy(out_i[:], ps_i[:, :N2])

            W0r = cp.tile([128, N2], f32, tag="W0r", name="W0r")
            W0i = cp.tile([128, N2], f32, tag="W0i", name="W0i")
            weight_dft(
                [(ct["ct1r"], ct["ct1i"], wc["w0r_c"], wc["w0i_c"])],
                (ct["te1r"], ct["te1i"], wc["w0r_e"], wc["w0i_e"]),
                W0r, W0i,
            )
            WLr = cp.tile([128, N2], f32, tag="WLr", name="WLr")
            WLi = cp.tile([128, N2], f32, tag="WLi", name="WLi")
            weight_dft(
                [(ct["ct1r"], ct["ct1i"], wc["wlr_c1"], wc["wli_c1"]),
                 (ct["ct2r"], ct["ct2i"], wc["wlr_c2"], wc["wli_c2"])],
                (ct["te2r"], ct["te2i"], wc["wlr_e"], wc["wli_e"]),
                WLr, WLi,
            )

            # ---- Cq = W0^2 * WL / (16N)  [128, 18] ----
            # Device evicts Zr16 = 16Zr and Zi16 = 16Zi (same scale), so all
            # three G variants are built from Cq = C/(16N).
            Cqr = cp.tile([128, N2], f32, tag="Cqr", name="Cqr")
            Cqi = cp.tile([128, N2], f32, tag="Cqi", name="Cqi")
            nCqr = cp.tile([128, N2], f32, tag="nCqr", name="nCqr")
            ta = tp.tile([128, N2], f32, tag="ca", name="ca")
            tb = tp.tile([128, N2], f32, tag="cb", name="cb")
            tm1 = tp.tile([128, N2], f32, tag="cm1", name="cm1")
            tm2 = tp.tile([128, N2], f32, tag="cm2", name="cm2")
            nc.vector.tensor_mul(tm1[:], W0r[:], W0r[:])
            nc.vector.tensor_mul(tm2[:], W0i[:], W0i[:])
            nc.vector.tensor_sub(ta[:], tm1[:], tm2[:])
            nc.vector.tensor_mul(tm1[:], W0r[:], W0i[:])
            nc.vector.tensor_add(tb[:], tm1[:], tm1[:])
            nc.vector.tensor_mul(tm1[:], ta[:], WLr[:])
            nc.vector.tensor_mul(tm2[:], tb[:], WLi[:])
            nc.vector.tensor_sub(tm1[:], tm1[:], tm2[:])
            nc.scalar.mul(Cqr[:], tm1[:], 1.0 / (16 * N))
            nc.scalar.mul(nCqr[:], tm1[:], -1.0 / (16 * N))
            nc.vector.tensor_mul(tm1[:], ta[:], WLi[:])
            nc.vector.tensor_mul(tm2[:], tb[:], WLr[:])
            nc.vector.tensor_add(tm1[:], tm1[:], tm2[:])
            nc.scalar.mul(Cqi[:], tm1[:], 1.0 / (16 * N))

            # ---- G variants (bf16): with inputs (Zr4, Zi4) = (4Zr, 4Zi):
            # Ur = Gqr.Zr4 + Gqni.Zi4 ; Ui = Gqi.Zr4 + Gqr.Zi4
            Gqr = cp.tile([128, N2 * 128], bf16, tag="Gqr", name="Gqr")
            Gqi = cp.tile([128, N2 * 128], bf16, tag="Gqi", name="Gqi")
            Gqni = cp.tile([128, N2 * 128], bf16, tag="Gqni", name="Gqni")

            def g_build(k2):
                sl = slice(k2 * 128, (k2 + 1) * 128)
                cqr_ = Cqr[:, k2:k2 + 1]
                cqi_ = Cqi[:, k2:k2 + 1]
                g1 = tp.tile([128, 128], f32, tag="g1", name="g1", bufs=2)
                nc.scalar.mul(g1[:], ct["cwii"], cqi_)
                nc.vector.scalar_tensor_tensor(Gqr[:, sl], ct["cwir"], cqr_, g1[:], AO.mult, AO.subtract)
                g2 = tp.tile([128, 128], f32, tag="g2", name="g2", bufs=2)
                nc.scalar.mul(g2[:], ct["cwir"], cqi_)
                nc.vector.scalar_tensor_tensor(Gqi[:, sl], ct["cwii"], cqr_, g2[:], AO.mult, AO.add)
                # Gqni = -(cwii*Cqr + cwir*Cqi) = (cwii * nCqr) - g2
                nc.vector.scalar_tensor_tensor(Gqni[:, sl], ct["cwii"], nCqr[:, k2:k2 + 1], g2[:], AO.mult, AO.subtract)

            # ---- F1: 9 pairs + 1 single ----
            Abig = bp.tile([128, PITCH_A], bf16, tag="big3", name="Abig")

            def f1_block(pr, pi, h, b):
                s, t = b % 2, b // 2
                K = 8 * FCNT[b]
                M = 18 * FCNT[b]
                rr = xt_r[64 * s: 64 * s + K, 512 * t: 512 * (t + 1)]
                ri = xt_i[64 * s: 64 * s + K, 512 * t: 512 * (t + 1)]
                lr = ct["cf1r"][64 * s: 64 * s + K, t * 126: t * 126 + M]
                li = ct["cf1i"][64 * s: 64 * s + K, t * 126: t * 126 + M]
                ln = ct["cf1n"][64 * s: 64 * s + K, t * 126: t * 126 + M]
                osl = slice(h * 512, (h + 1) * 512)
                nc.tensor.matmul(pr[:M, osl], lr, rr, start=True, stop=False)
                nc.tensor.matmul(pr[:M, osl], ln, ri, start=False, stop=True)
                nc.tensor.matmul(pi[:M, osl], li, rr, start=True, stop=False)
                nc.tensor.matmul(pi[:M, osl], lr, ri, start=False, stop=True)

            def f1_pivot(ag, h, b, npart):
                # ag partition = il*18+k2 ; cols (plane, blkhalf, b)
                cnt = FCNT[b]
                in_ap = bass.AP(ag.tensor, ag[:].offset + h * 512,
                                [[2048, cnt * 18], [1024, 2], [1, 512]])
                out_ap = bass.AP(Abig.tensor, Abig[:].offset + (7 * b) * PITCH_A,
                                 [[PITCH_A, cnt], [1024, 18], [512, 2], [1, 512]])
                nc.scalar.dma_start(out=out_ap, in_=in_ap)

            for p in range(10):
                blocks = [2 * p] if p == 9 else [2 * p, 2 * p + 1]
                pr2 = psa.tile([128, 1024], f32, tag="pA", name="pr2")
                pi2 = psa.tile([128, 1024], f32, tag="pB", name="pi2")
                for h, b in enumerate(blocks):
                    f1_block(pr2, pi2, h, b)
                if p == 9:
                    ag = sp.tile([36, 2048], bf16, tag="ag", name="ag")
                    nc.scalar.activation(ag[:36, 0:512], pr2[:36, 0:512], AF.Copy)
                    nc.vector.tensor_copy(ag[:36, 1024:1536], pi2[:36, 0:512])
                    f1_pivot(ag, 0, 18, 36)
                else:
                    ag = sp.tile([126, 2048], bf16, tag="ag", name="ag")
                    nc.scalar.activation(ag[:, 0:1024], pr2[:126, :], AF.Copy)
                    nc.vector.tensor_copy(ag[:, 1024:2048], pi2[:126, :])
                    f1_pivot(ag, 0, 2 * p, 126)
                    f1_pivot(ag, 1, 2 * p + 1, 126)
                if 2 * p < N2:
                    g_build(2 * p)
                if 2 * p + 1 < N2:
                    g_build(2 * p + 1)

            # ---- F3 + square + I1 (I1 runs one pair behind F3), 9 k2-pairs ----
            Zr4 = bp.tile([128, N2 * 512], bf16, tag="big2", name="Zr4")   # aliases xt_i
            Zi4 = bp.tile([128, N2 * 512], bf16, tag="big4", name="Zi4")
            Ubig = bp.tile([128, PITCH_U], bf16, tag="big1", name="Ubig")  # aliases xt_r

            def f3_pair(q):
                Xr2 = psa.tile([128, 1024], f32, tag="pA", name="Xr2")
                Xi2 = psa.tile([128, 1024], f32, tag="pB", name="Xi2")
                for h in range(2):
                    k2 = 2 * q + h
                    a_r = Abig[:, k2 * 1024: k2 * 1024 + 512]
                    a_i = Abig[:, k2 * 1024 + 512: (k2 + 1) * 1024]
                    osl = slice(h * 512, (h + 1) * 512)
                    nc.tensor.matmul(Xr2[:, osl], ct["cw3r"], a_r, start=True, stop=False)
                    nc.tensor.matmul(Xr2[:, osl], ct["cw3n"], a_i, start=False, stop=True)
                    nc.tensor.matmul(Xi2[:, osl], ct["cw3i"], a_r, start=True, stop=False)
                    nc.tensor.matmul(Xi2[:, osl], ct["cw3r"], a_i, start=False, stop=True)
                zsl = slice(q * 1024, (q + 1) * 1024)
                xr4 = tp.tile([128, 1024], bf16, tag="xr4", name="xr4")
                nc.scalar.activation(xr4[:], Xr2[:], AF.Copy, scale=4.0)       # 4Xr
                xi4 = tp.tile([128, 1024], bf16, tag="xi4", name="xi4")
                nc.scalar.activation(xi4[:], Xi2[:], AF.Copy, scale=4.0)       # 4Xi
                m1 = tp.tile([128, 1024], bf16, tag="m1", name="m1")
                nc.vector.tensor_mul(m1[:], xr4[:], xr4[:])                    # 16Xr^2
                m2 = tp.tile([128, 1024], bf16, tag="m2", name="m2")
                nc.vector.tensor_mul(m2[:], xi4[:], xi4[:])                    # 16Xi^2
                nc.gpsimd.tensor_sub(Zr4[:, zsl], m1[:], m2[:])                # 16Zr
                # Zi16 = (2*4Xr) * (4Xi) = 32XrXi = 16*(2XrXi) = 16Zi
                nc.vector.scalar_tensor_tensor(Zi4[:, zsl], xr4[:], 2.0, xi4[:], AO.mult, AO.mult)

            def i1_pair(q):
                Ur2 = psa.tile([128, 1024], f32, tag="pA", name="Ur2")
                Ui2 = psa.tile([128, 1024], f32, tag="pB", name="Ui2")
                for h in range(2):
                    k2 = 2 * q + h
                    gsl = slice(k2 * 128, (k2 + 1) * 128)
                    zs = slice(q * 1024 + h * 512, q * 1024 + (h + 1) * 512)
                    osl = slice(h * 512, (h + 1) * 512)
                    nc.tensor.matmul(Ur2[:, osl], Gqr[:, gsl], Zr4[:, zs], start=True, stop=False)
                    nc.tensor.matmul(Ur2[:, osl], Gqni[:, gsl], Zi4[:, zs], start=False, stop=True)
                    nc.tensor.matmul(Ui2[:, osl], Gqi[:, gsl], Zr4[:, zs], start=True, stop=False)
                    nc.tensor.matmul(Ui2[:, osl], Gqr[:, gsl], Zi4[:, zs], start=False, stop=True)
                u_out_r = bass.AP(Ubig.tensor, Ubig[:].offset + (2 * q) * 1024,
                                  [[PITCH_U, 128], [1024, 2], [1, 512]])
                u_out_i = bass.AP(Ubig.tensor, Ubig[:].offset + (2 * q) * 1024 + 512,
                                  [[PITCH_U, 128], [1024, 2], [1, 512]])
                nc.scalar.activation(u_out_r, Ur2[:], AF.Copy)
                nc.vector.tensor_copy(u_out_i, Ui2[:])

            for q in range(9):
                f3_pair(q)
                if q > 0:
                    i1_pair(q - 1)
            i1_pair(8)
            if DEBUG_DUMP:
                nc.sync.dma_start(out=dbg["dAbig"][:, :], in_=Abig[:])
                nc.sync.dma_start(out=dbg["dxt"][:, :], in_=xt_r[:])

            # ---- pivot-D (DVE queue, prefetched) + I2, 9 pairs + 1 single ----
            def pivot_d(b):
                cnt = FCNT[b]
                u2 = sp.tile([126, 1024], bf16, tag="u2", name="u2", bufs=4)
                in_ap = bass.AP(Ubig.tensor, Ubig[:].offset + (7 * b) * PITCH_U,
                                [[PITCH_U, cnt], [1, N2 * 1024]])
                out_ap = bass.AP(u2.tensor, u2[:].offset,
                                 [[1024, cnt * 18], [1, 1024]])
                nc.sync.dma_start(out=out_ap, in_=in_ap)
                return u2

            sreg = bp.tile([112, 8192], bf16, tag="big3", name="sreg")
            u2q = [pivot_d(0), pivot_d(1)]
            for p in range(10):
                blocks = [2 * p] if p == 9 else [2 * p, 2 * p + 1]
                u2s = [u2q.pop(0) for _ in blocks]
                for b in range(2 * p + 2, min(2 * p + 4, NB)):
                    u2q.append(pivot_d(b))
                yr2 = psa.tile([128, 1024], f32, tag="pA", name="yr2")
                yi2 = psa.tile([128, 1024], f32, tag="pB", name="yi2")
                for h, b in enumerate(blocks):
                    K = 18 * FCNT[b]
                    M = 16 * FCNT[b]
                    off = b * 112
                    osl = slice(h * 512, (h + 1) * 512)
                    u2 = u2s[h]
                    nc.tensor.matmul(yr2[:M, osl], ct["ci2r"][:K, off:off + M], u2[:K, 0:512], start=True, stop=False)
                    nc.tensor.matmul(yr2[:M, osl], ct["ci2n"][:K, off:off + M], u2[:K, 512:1024], start=False, stop=True)
                    nc.tensor.matmul(yi2[:M, osl], ct["ci2i"][:K, off:off + M], u2[:K, 0:512], start=True, stop=False)
                    nc.tensor.matmul(yi2[:M, osl], ct["ci2r"][:K, off:off + M], u2[:K, 512:1024], start=False, stop=True)
                Mp = 32 if p == 9 else 112
                W = 512 if p == 9 else 1024
                sb = (p % 2) * 4096
                s1 = sreg[:, sb + 0: sb + 1024]
                s2 = sreg[:, sb + 1024: sb + 2048]
                ss = sreg[:, sb + 2048: sb + 3072]
                yis = sreg[:, sb + 3072: sb + 4096]
                nc.scalar.activation(s1[:Mp, :W], yr2[:Mp, :W], AF.Square)
                nc.vector.tensor_copy(yis[:Mp, :W], yi2[:Mp, :W])
                nc.vector.tensor_mul(s2[:Mp, :W], yis[:Mp, :W], yis[:Mp, :W])
                nc.vector.tensor_add(ss[:Mp, :W], s1[:Mp, :W], s2[:Mp, :W])
                ya = sp.tile([112, 1024], bf16, tag="ya", name="ya", bufs=2)
                nc.scalar.activation(ya[:Mp, :W], ss[:Mp, :W], AF.Sqrt)
                if p == 9:
                    nc.scalar.dma_start(out=yraw[18 * 112: 18 * 112 + 32, :], in_=ya[:32, :512])
                else:
                    nc.scalar.dma_start(out=yraw[2 * p * 112: (2 * p + 1) * 112, :], in_=ya[:112, 0:512])
                    nc.scalar.dma_start(out=yraw[(2 * p + 1) * 112: (2 * p + 2) * 112, :], in_=ya[:112, 512:1024])

            if DEBUG_DUMP:
                nc.sync.dma_start(out=dbg["dZr"][:, :], in_=Zr4[:])
                nc.sync.dma_start(out=dbg["dZi"][:, :], in_=Zi4[:])
                nc.sync.dma_start(out=dbg["dU"][:, :], in_=Ubig[:])

    nc.compile()
    return nc


_NC_CACHE = None


def _prep_inputs(x_real, x_imag):
    """Pure data movement: permute+transpose x into the [1280, B] device layout."""
    valid = XRMAP >= 0
    out = []
    for x in (x_real, x_imag):
        xt = np.zeros((XROWS, B), np.float32)
        xt[valid] = np.ascontiguousarray(x[:, XRMAP[valid]].T)
        out.append(xt)
    return out


def kernel(**inputs):
    global _NC_CACHE
    x_real = np.ascontiguousarray(inputs["x_real"], dtype=np.float32)
    x_imag = np.ascontiguousarray(inputs["x_imag"], dtype=np.float32)
    w0_real = np.ascontiguousarray(inputs["w0_real"], dtype=np.float32)
    w0_imag = np.ascontiguousarray(inputs["w0_imag"], dtype=np.float32)
    wl_real = np.ascontiguousarray(inputs["wl_real"], dtype=np.float32)
    wl_imag = np.ascontiguousarray(inputs["wl_imag"], dtype=np.float32)

    xtr_full, xti_full = _prep_inputs(x_real, x_imag)

    wpack = np.stack([w0_real[0:128], w0_imag[0:128],
                      wl_real[0:128], wl_imag[0:128],
                      wl_real[128:256], wl_imag[128:256]], axis=1).astype(np.float32)
    wtail = np.array([[w0_real[128], w0_imag[128], wl_real[256], wl_imag[256]]],
                     np.float32)
    const_maps = {nm: np.ascontiguousarray(arr) for nm, arr in CONSTS.items()}
    in_maps = []
    for cid in range(NCORES):
        cols = slice(cid * BCORE, (cid + 1) * BCORE)
        m = {
            "xtr": np.ascontiguousarray(xtr_full[:, cols]),
            "xti": np.ascontiguousarray(xti_full[:, cols]),
            "wpack": wpack, "wtail": wtail,
        }
        m.update(const_maps)
        in_maps.append(m)

    if _NC_CACHE is None:
        _NC_CACHE = build_nc()
    res = run_bass_kernel_spmd(_NC_CACHE, in_maps, core_ids=list(range(NCORES)))

    out = np.empty((B, CLASS_NUM), np.float32)
    for cid in range(NCORES):
        yr = np.asarray(res.results[cid]["yraw"]).astype(np.float32)  # [2048, 512]
        out[cid * BCORE:(cid + 1) * BCORE, :] = yr[YINV, :].T
    return out


# revision 24
# speedup vs baseline: 1.7151x; 1.0756x over previous
"""Trainium2 Bass kernel for nn_CNN_Comp_29240137351522 (dense_cnn).

Math:  y = |IFFT_N( FFT_N(x)^2 * C )[255:2303]|,  C = FFT_N(w0)^2 * FFT_N(wl) / N
with N = 2304 = 128*18.  2304 >= 2303-ish: circular aliasing only contaminates
samples n < 255, which the center crop [255, 2303) never reads, so the chained
full convolutions are exact on the cropped output.

Device decomposition (per core, data-parallel over batch, b = 512 samples):
  N = N1*N2, N1=128, N2=18;  time n = n2*128+n1,  freq k = k1*18+k2
  F1 (contract n2<8, PE, 19 blocks of <=7 n1-values, twiddle folded)
  F3 (contract n1, PE, shared W128 DFT, bf16)      -> X[k1, (k2,b)]
  square (ACT/DVE/Pool fused eviction)             -> Zr4 = 4(Xr^2-Xi^2), Zi2 = 4XrXi
  I1 (contract k1, PE, per-k2 G = (C/4N)-row-scaled inverse DFT), bf16
  I2 (contract k2, PE, 19 blocks of <=7 n1'-values, twiddle folded,
      exactly 16 valid n2' outputs per n1' -> 2048 rows), bf16
  |.| fused into I2 eviction; raw bf16 tiles stored to DRAM, unscrambled on host.

Host does data movement only: batch shard, pre-transposed/permuted copy of x
(so no on-device transposes are needed), and the inverse row->output-column
unscramble of the raw result.
"""

import numpy as np
import ml_dtypes

import concourse.bass as bass
import concourse.bacc as bacc
import concourse.mybir as mybir
from concourse.tile import TileContext
from concourse.bass_utils import run_bass_kernel_spmd

# ---------------- static problem config ----------------
B, NX = 4096, 1024
K0, KL = 129, 257
N = 2304
N1, N2 = 128, 18
NCORES = 8
BCORE = B // NCORES          # 512
FCNT = [7] * 18 + [2]        # il-count per block; n1 = 7*b + il
NB = 19
CROP0 = 255
CLASS_NUM = 2048
XROWS = 1280                 # 10 tiles of 128 rows, 2 blocks per tile
PITCH_A = N2 * 1024          # Abig free size  (k2, plane, b)
PITCH_U = N2 * 1024          # Ubig free size  (k2, plane, b)

f32 = mybir.dt.float32
f32r = mybir.dt.float32r
bf16 = mybir.dt.bfloat16
AO = mybir.AluOpType
AF = mybir.ActivationFunctionType


def _w(num, den):
    return np.exp(-2j * np.pi * np.asarray(num, np.float64) / den)


# ---------------- host-side constant arrays ----------------
def _build_consts():
    c = {}
    k1g = np.arange(N1)
    k2g = np.arange(N2)
    nh = np.arange(128)

    # F1 lhsT: [112, 10*126]; pair slot s=b%2 at rows [56s,56s+56);
    # block b at cols [(b//2)*126, ...); rows (il)*8+n2, cols il*18+k2;
    # value W18[n2,k2] * W2304^{n1 k2}, n1 = 7b+il
    f1 = np.zeros((120, 10 * 126), np.complex128)
    n2g8 = np.arange(8)
    for b in range(NB):
        s, t = b % 2, b // 2
        for il in range(FCNT[b]):
            n1 = 7 * b + il
            blk = _w(np.outer(n2g8, k2g), N2) * _w(n1 * k2g, N)[None, :]
            f1[64 * s + il * 8: 64 * s + il * 8 + 8,
               t * 126 + il * 18: t * 126 + (il + 1) * 18] = blk
    c["cf1r"] = f1.real.astype(ml_dtypes.bfloat16)
    c["cf1i"] = f1.imag.astype(ml_dtypes.bfloat16)
    c["cf1n"] = (-f1.imag).astype(ml_dtypes.bfloat16)

    # F3 lhsT (shared, bf16): W128[n1,k1]
    w3 = _w(np.outer(nh, k1g), N1)
    c["cw3r"] = w3.real.astype(ml_dtypes.bfloat16)
    c["cw3i"] = w3.imag.astype(ml_dtypes.bfloat16)
    c["cw3n"] = (-w3.imag).astype(ml_dtypes.bfloat16)
    # f32 copies for the weight DFT (precision)
    c["cw3rf"] = w3.real.astype(np.float32)
    c["cw3if"] = w3.imag.astype(np.float32)
    c["cw3nf"] = (-w3.imag).astype(np.float32)

    # I1 base: W128i[k1,n1] (f32, G built on device)
    wi = _w(-np.outer(k1g, np.arange(N1)), N1)
    c["cwir"] = wi.real.astype(np.float32)
    c["cwii"] = wi.imag.astype(np.float32)

    # I2 lhsT: [126, 2048]; block b cols [b*112, b*112+16*cnt);
    # rows il*18+k2, cols il*16+j; n2'(j) = (1 if n1'==127 else 2)+j
    i2 = np.zeros((126, 2048), np.complex128)
    for b in range(NB):
        for il in range(FCNT[b]):
            n1p = 7 * b + il
            base = 1 if n1p == 127 else 2
            n2p = base + np.arange(16)
            blk = _w(-np.outer(k2g, n2p), N2) * _w(-n1p * k2g, N)[:, None]
            i2[il * 18: (il + 1) * 18,
               b * 112 + il * 16: b * 112 + (il + 1) * 16] = blk
    c["ci2r"] = i2.real.astype(ml_dtypes.bfloat16)
    c["ci2i"] = i2.imag.astype(ml_dtypes.bfloat16)
    c["ci2n"] = (-i2.imag).astype(ml_dtypes.bfloat16)

    # weight-DFT rhs constants
    t129 = _w(np.outer(nh, k2g), N)
    c["ct1r"] = t129.real.astype(np.float32)
    c["ct1i"] = t129.imag.astype(np.float32)
    te1 = _w(k2g, N2)
    t257b = t129 * te1[None, :]
    c["ct2r"] = t257b.real.astype(np.float32)
    c["ct2i"] = t257b.imag.astype(np.float32)
    c["te1r"] = te1.real.astype(np.float32).reshape(1, N2)
    c["te1i"] = te1.imag.astype(np.float32).reshape(1, N2)
    te2 = _w(k2g, 9)
    c["te2r"] = te2.real.astype(np.float32).reshape(1, N2)
    c["te2i"] = te2.imag.astype(np.float32).reshape(1, N2)

    c["ones1"] = np.ones((1, 128), np.float32)

    # ---- pack into few DRAM tensors (fewer DMAs; device slices views) ----
    p = {}
    p["wdftf"] = np.concatenate([c[k] for k in
        ["ct1r", "ct1i", "ct2r", "ct2i", "cw3rf", "cw3if", "cw3nf"]], axis=1)
    p["smalls"] = np.concatenate([c[k] for k in
        ["te1r", "te1i", "te2r", "te2i", "ones1"]], axis=1)
    p["cwaux"] = np.concatenate([c["cwir"], c["cwii"]], axis=1)
    p["cf1p"] = np.concatenate([c["cf1r"], c["cf1i"], c["cf1n"]], axis=1)
    p["cw3p"] = np.concatenate([c["cw3r"], c["cw3i"], c["cw3n"]], axis=1)
    p["ci2p"] = np.concatenate([c["ci2r"], c["ci2i"], c["ci2n"]], axis=1)
    return p


CONSTS = _build_consts()


def _xrow_map():
    """DRAM row r of xp_t -> original column n of x (or -1 for pad)."""
    rmap = np.full(XROWS, -1, np.int64)
    for r in range(XROWS):
        t, rr = divmod(r, 128)
        s, rrr = divmod(rr, 64)
        if rrr >= 56:
            continue
        il, n2 = divmod(rrr, 8)
        b = 2 * t + s
        if b >= NB or il >= FCNT[b]:
            continue
        n1 = 7 * b + il
        rmap[r] = n2 * 128 + n1
    return rmap


def _ycol_map():
    """yraw row r -> output column (n - 255)."""
    cols = np.empty(2048, np.int64)
    r = 0
    for b in range(NB):
        for il in range(FCNT[b]):
            n1p = 7 * b + il
            base = 1 if n1p == 127 else 2
            for j in range(16):
                cols[b * 112 + il * 16 + j] = (base + j) * 128 + n1p - CROP0
                r += 1
    return cols


XRMAP = _xrow_map()
YCOLS = _ycol_map()
YINV = np.empty(2048, np.int64)
YINV[YCOLS] = np.arange(2048)


# ---------------- bass kernel builder ----------------
DEBUG_DUMP = False


def build_nc():
    nc = bacc.Bacc("TRN2", target_bir_lowering=False, debug=False, num_devices=NCORES)

    d = {}
    d["xtr"] = nc.dram_tensor("xtr", [XROWS, BCORE], bf16, kind="ExternalInput")
    d["xti"] = nc.dram_tensor("xti", [XROWS, BCORE], bf16, kind="ExternalInput")
    d["wpack"] = nc.dram_tensor("wpack", [128, 6], f32, kind="ExternalInput")
    d["wtail"] = nc.dram_tensor("wtail", [1, 4], f32, kind="ExternalInput")
    cdt = {"cf1p": bf16, "cw3p": bf16, "ci2p": bf16}
    for nm, arr in CONSTS.items():
        d[nm] = nc.dram_tensor(nm, list(arr.shape), cdt.get(nm, f32), kind="ExternalInput")
    yraw = nc.dram_tensor("yraw", [2048, BCORE], bf16, kind="ExternalOutput")
    dbg = {}
    if DEBUG_DUMP:
        dbg["dAbig"] = nc.dram_tensor("dAbig", [128, N2 * 1024], bf16, kind="ExternalOutput")
        dbg["dZr"] = nc.dram_tensor("dZr", [128, N2 * 512], bf16, kind="ExternalOutput")
        dbg["dZi"] = nc.dram_tensor("dZi", [128, N2 * 512], bf16, kind="ExternalOutput")
        dbg["dU"] = nc.dram_tensor("dU", [128, N2 * 1024], bf16, kind="ExternalOutput")
        dbg["dxt"] = nc.dram_tensor("dxt", [128, 10 * BCORE], bf16, kind="ExternalOutput")

    with TileContext(nc) as tc:
        with (
            tc.tile_pool(name="cp", bufs=1) as cp,          # consts + persistent
            tc.tile_pool(name="bp", bufs=1) as bp,          # big tiles (tag-aliased)
            tc.tile_pool(name="sp", bufs=2) as sp,          # rotating bf16 tiles
            tc.tile_pool(name="tp", bufs=2) as tp,          # rotating f32 tmps
            tc.tile_pool(name="psa", bufs=2, space="PSUM") as psa,  # 2 tags x 2 bufs x 2 banks
        ):
            # ---- load packed weights/consts (few DMAs; critical-path first) ----
            wpk = cp.tile([128, 6], f32, tag="wpk", name="wpk")
            nc.sync.dma_start(out=wpk[:], in_=d["wpack"][:, :])
            wtl = cp.tile([1, 4], f32, tag="wtl", name="wtl")
            nc.sync.dma_start(out=wtl[:], in_=d["wtail"][:, :])
            wc = {"w0r_c": wpk[:, 0:1], "w0i_c": wpk[:, 1:2],
                  "wlr_c1": wpk[:, 2:3], "wli_c1": wpk[:, 3:4],
                  "wlr_c2": wpk[:, 4:5], "wli_c2": wpk[:, 5:6],
                  "w0r_e": wtl[:, 0:1], "w0i_e": wtl[:, 1:2],
                  "wlr_e": wtl[:, 2:3], "wli_e": wtl[:, 3:4]}

            pk = {}
            for nm, eng in [("wdftf", nc.sync), ("smalls", nc.sync), ("cf1p", nc.sync),
                            ("cwaux", nc.scalar), ("cw3p", nc.scalar), ("ci2p", nc.scalar)]:
                arr = CONSTS[nm]
                t = cp.tile(list(arr.shape), cdt.get(nm, f32), tag=nm, name=nm)
                eng.dma_start(out=t[:], in_=d[nm][:, :])
                pk[nm] = t
            ct = {
                "ct1r": pk["wdftf"][:, 0:18], "ct1i": pk["wdftf"][:, 18:36],
                "ct2r": pk["wdftf"][:, 36:54], "ct2i": pk["wdftf"][:, 54:72],
                "cw3rf": pk["wdftf"][:, 72:200], "cw3if": pk["wdftf"][:, 200:328],
                "cw3nf": pk["wdftf"][:, 328:456],
                "te1r": pk["smalls"][:1, 0:18], "te1i": pk["smalls"][:1, 18:36],
                "te2r": pk["smalls"][:1, 36:54], "te2i": pk["smalls"][:1, 54:72],
                "ones1": pk["smalls"][:1, 72:200],
                "cwir": pk["cwaux"][:, 0:128], "cwii": pk["cwaux"][:, 128:256],
                "cf1r": pk["cf1p"][:, 0:1260], "cf1i": pk["cf1p"][:, 1260:2520],
                "cf1n": pk["cf1p"][:, 2520:3780],
                "cw3r": pk["cw3p"][:, 0:128], "cw3i": pk["cw3p"][:, 128:256],
                "cw3n": pk["cw3p"][:, 256:384],
                "ci2r": pk["ci2p"][:, 0:2048], "ci2i": pk["ci2p"][:, 2048:4096],
                "ci2n": pk["ci2p"][:, 4096:6144],
            }

            # ---- load x planes in 4 chunks per plane (F1 starts early) ----
            xt_r = bp.tile([128, 10 * BCORE], bf16, tag="big1", name="xt_r")
            xt_i = bp.tile([128, 10 * BCORE], bf16, tag="big2", name="xt_i")
            for c0, ntile in ((0, 3), (3, 3), (6, 3), (9, 1)):
                for xt, srcn in ((xt_r, "xtr"), (xt_i, "xti")):
                    nc.sync.dma_start(
                        out=xt[:, 512 * c0: 512 * (c0 + ntile)],
                        in_=bass.AP(d[srcn][:, :].tensor, 128 * c0 * BCORE,
                                    [[BCORE, 128], [128 * BCORE, ntile], [1, BCORE]]),
                    )

            # ---- weight DFT: W0, WL [128, 18] ----
            def build_rhs(tr, ti, cr_, ci_, out_r, out_i):
                tmp = tp.tile([tr.shape[0], N2], f32, tag="wtmp", name="wtmp")
                nc.vector.tensor_scalar(tmp[:], ti, ci_, None, AO.mult)
                nc.vector.scalar_tensor_tensor(out_r[:], tr, cr_, tmp[:], AO.mult, AO.subtract)
                tmp2 = tp.tile([tr.shape[0], N2], f32, tag="wtmp2", name="wtmp2")
                nc.vector.tensor_scalar(tmp2[:], tr, ci_, None, AO.mult)
                nc.vector.scalar_tensor_tensor(out_i[:], ti, cr_, tmp2[:], AO.mult, AO.add)

            def weight_dft(chunks, tail, out_r, out_i):
                ps_r = psa.tile([128, 1024], f32, tag="pA", name="ps_r")
                ps_i = psa.tile([128, 1024], f32, tag="pB", name="ps_i")
                rhs = []
                for (t_r, t_i, colr, coli) in chunks:
                    rr = sp.tile([128, N2], f32, tag="wrhs_r", name="wrhs_r")
                    ri = sp.tile([128, N2], f32, tag="wrhs_i", name="wrhs_i")
                    build_rhs(t_r, t_i, colr, coli, rr, ri)
                    rhs.append((rr, ri))
                te_r, te_i, er, ei = tail
                tr_ = sp.tile([1, N2], f32, tag="wtail_r", name="wtail_r")
                ti_ = sp.tile([1, N2], f32, tag="wtail_i", name="wtail_i")
                tmp = tp.tile([1, N2], f32, tag="wtmp3", name="wtmp3")
                nc.vector.tensor_scalar(tmp[:], te_i, ei, None, AO.mult)
                nc.vector.scalar_tensor_tensor(tr_[:], te_r, er, tmp[:], AO.mult, AO.subtract)
                tmp2 = tp.tile([1, N2], f32, tag="wtmp4", name="wtmp4")
                nc.vector.tensor_scalar(tmp2[:], te_r, ei, None, AO.mult)
                nc.vector.scalar_tensor_tensor(ti_[:], te_i, er, tmp2[:], AO.mult, AO.add)
                first = True
                for (rr, ri) in rhs:
                    nc.tensor.matmul(ps_r[:, :N2], ct["cw3rf"], rr[:], start=first, stop=False)
                    nc.tensor.matmul(ps_r[:, :N2], ct["cw3nf"], ri[:], start=False, stop=False)
                    first = False
                nc.tensor.matmul(ps_r[:, :N2], ct["ones1"], tr_[:], start=False, stop=True)
                first = True
                for (rr, ri) in rhs:
                    nc.tensor.matmul(ps_i[:, :N2], ct["cw3if"], rr[:], start=first, stop=False)
                    nc.tensor.matmul(ps_i[:, :N2], ct["cw3rf"], ri[:], start=False, stop=False)
                    first = False
                nc.tensor.matmul(ps_i[:, :N2], ct["ones1"], ti_[:], start=False, stop=True)
                nc.vector.tensor_copy(out_r[:], ps_r[:, :N2])
                nc.vector.tensor_copy(out_i[:], ps_i[:, :N2])

            W0r = cp.tile([128, N2], f32, tag="W0r", name="W0r")
            W0i = cp.tile([128, N2], f32, tag="W0i", name="W0i")
            weight_dft(
                [(ct["ct1r"], ct["ct1i"], wc["w0r_c"], wc["w0i_c"])],
                (ct["te1r"], ct["te1i"], wc["w0r_e"], wc["w0i_e"]),
                W0r, W0i,
            )
            WLr = cp.tile([128, N2], f32, tag="WLr", name="WLr")
            WLi = cp.tile([128, N2], f32, tag="WLi", name="WLi")
            weight_dft(
                [(ct["ct1r"], ct["ct1i"], wc["wlr_c1"], wc["wli_c1"]),
                 (ct["ct2r"], ct["ct2i"], wc["wlr_c2"], wc["wli_c2"])],
                (ct["te2r"], ct["te2i"], wc["wlr_e"], wc["wli_e"]),
                WLr, WLi,
            )

            # ---- Cq = W0^2 * WL / (16N)  [128, 18] ----
            # Device evicts Zr16 = 16Zr and Zi16 = 16Zi (same scale), so all
            # three G variants are built from Cq = C/(16N).
            Cqr = cp.tile([128, N2], f32, tag="Cqr", name="Cqr")
            Cqi = cp.tile([128, N2], f32, tag="Cqi", name="Cqi")
            nCqr = cp.tile([128, N2], f32, tag="nCqr", name="nCqr")
            ta = tp.tile([128, N2], f32, tag="ca", name="ca")
            tb = tp.tile([128, N2], f32, tag="cb", name="cb")
            tm1 = tp.tile([128, N2], f32, tag="cm1", name="cm1")
            tm2 = tp.tile([128, N2], f32, tag="cm2", name="cm2")
            nc.vector.tensor_mul(tm1[:], W0r[:], W0r[:])
            nc.vector.tensor_mul(tm2[:], W0i[:], W0i[:])
            nc.vector.tensor_sub(ta[:], tm1[:], tm2[:])
            nc.vector.tensor_mul(tm1[:], W0r[:], W0i[:])
            nc.vector.tensor_add(tb[:], tm1[:], tm1[:])
            nc.vector.tensor_mul(tm1[:], ta[:], WLr[:])
            nc.vector.tensor_mul(tm2[:], tb[:], WLi[:])
            nc.vector.tensor_sub(tm1[:], tm1[:], tm2[:])
            nc.scalar.mul(Cqr[:], tm1[:], 1.0 / (16 * N))
            nc.scalar.mul(nCqr[:], tm1[:], -1.0 / (16 * N))
            nc.vector.tensor_mul(tm1[:], ta[:], WLi[:])
            nc.vector.tensor_mul(tm2[:], tb[:], WLr[:])
            nc.vector.tensor_add(tm1[:], tm1[:], tm2[:])
            nc.scalar.mul(Cqi[:], tm1[:], 1.0 / (16 * N))

            # ---- G variants (bf16): with inputs (Zr4, Zi4) = (4Zr, 4Zi):
            # Ur = Gqr.Zr4 + Gqni.Zi4 ; Ui = Gqi.Zr4 + Gqr.Zi4
            Gqr = cp.tile([128, N2 * 128], bf16, tag="Gqr", name="Gqr")
            Gqi = cp.tile([128, N2 * 128], bf16, tag="Gqi", name="Gqi")
            Gqni = cp.tile([128, N2 * 128], bf16, tag="Gqni", name="Gqni")

            def g_build(k2):
                sl = slice(k2 * 128, (k2 + 1) * 128)
                cqr_ = Cqr[:, k2:k2 + 1]
                cqi_ = Cqi[:, k2:k2 + 1]
                g1 = tp.tile([128, 128], f32, tag="g1", name="g1", bufs=2)
                nc.scalar.mul(g1[:], ct["cwii"], cqi_)
                nc.vector.scalar_tensor_tensor(Gqr[:, sl], ct["cwir"], cqr_, g1[:], AO.mult, AO.subtract)
                g2 = tp.tile([128, 128], f32, tag="g2", name="g2", bufs=2)
                nc.scalar.mul(g2[:], ct["cwir"], cqi_)
                nc.vector.scalar_tensor_tensor(Gqi[:, sl], ct["cwii"], cqr_, g2[:], AO.mult, AO.add)
                # Gqni = -(cwii*Cqr + cwir*Cqi) = (cwii * nCqr) - g2
                nc.vector.scalar_tensor_tensor(Gqni[:, sl], ct["cwii"], nCqr[:, k2:k2 + 1], g2[:], AO.mult, AO.subtract)

            # ---- F1: 9 pairs + 1 single ----
            Abig = bp.tile([128, PITCH_A], bf16, tag="big3", name="Abig")

            def f1_block(pr, pi, h, b):
                s, t = b % 2, b // 2
                K = 8 * FCNT[b]
                M = 18 * FCNT[b]
                rr = xt_r[64 * s: 64 * s + K, 512 * t: 512 * (t + 1)]
                ri = xt_i[64 * s: 64 * s + K, 512 * t: 512 * (t + 1)]
                lr = ct["cf1r"][64 * s: 64 * s + K, t * 126: t * 126 + M]
                li = ct["cf1i"][64 * s: 64 * s + K, t * 126: t * 126 + M]
                ln = ct["cf1n"][64 * s: 64 * s + K, t * 126: t * 126 + M]
                osl = slice(h * 512, (h + 1) * 512)
                nc.tensor.matmul(pr[:M, osl], lr, rr, start=True, stop=False)
                nc.tensor.matmul(pr[:M, osl], ln, ri, start=False, stop=True)
                nc.tensor.matmul(pi[:M, osl], li, rr, start=True, stop=False)
                nc.tensor.matmul(pi[:M, osl], lr, ri, start=False, stop=True)

            def f1_pivot(ag, h, b, npart):
                # ag partition = il*18+k2 ; cols (plane, blkhalf, b)
                cnt = FCNT[b]
                in_ap = bass.AP(ag.tensor, ag[:].offset + h * 512,
                                [[2048, cnt * 18], [1024, 2], [1, 512]])
                out_ap = bass.AP(Abig.tensor, Abig[:].offset + (7 * b) * PITCH_A,
                                 [[PITCH_A, cnt], [1024, 18], [512, 2], [1, 512]])
                nc.sync.dma_start(out=out_ap, in_=in_ap)

            for p in range(10):
                blocks = [2 * p] if p == 9 else [2 * p, 2 * p + 1]
                pr2 = psa.tile([128, 1024], f32, tag="pA", name="pr2")
                pi2 = psa.tile([128, 1024], f32, tag="pB", name="pi2")
                for h, b in enumerate(blocks):
                    f1_block(pr2, pi2, h, b)
                if p == 9:
                    ag = sp.tile([36, 2048], bf16, tag="ag", name="ag")
                    nc.scalar.activation(ag[:36, 0:512], pr2[:36, 0:512], AF.Copy)
                    nc.vector.tensor_copy(ag[:36, 1024:1536], pi2[:36, 0:512])
                    f1_pivot(ag, 0, 18, 36)
                else:
                    ag = sp.tile([126, 2048], bf16, tag="ag", name="ag")
                    nc.scalar.activation(ag[:, 0:1024], pr2[:126, :], AF.Copy)
                    nc.vector.tensor_copy(ag[:, 1024:2048], pi2[:126, :])
                    f1_pivot(ag, 0, 2 * p, 126)
                    f1_pivot(ag, 1, 2 * p + 1, 126)
                if 2 * p < N2:
                    g_build(2 * p)
                if 2 * p + 1 < N2:
                    g_build(2 * p + 1)

            # ---- F3 + square + I1 (I1 runs one pair behind F3), 9 k2-pairs ----
            Zr4 = bp.tile([128, N2 * 512], bf16, tag="big2", name="Zr4")   # aliases xt_i
            Zi4 = bp.tile([128, N2 * 512], bf16, tag="big4", name="Zi4")
            Ubig = bp.tile([128, PITCH_U], bf16, tag="big1", name="Ubig")  # aliases xt_r

            def f3_pair(q):
                Xr2 = psa.tile([128, 1024], f32, tag="pA", name="Xr2")
                Xi2 = psa.tile([128, 1024], f32, tag="pB", name="Xi2")
                for h in range(2):
                    k2 = 2 * q + h
                    a_r = Abig[:, k2 * 1024: k2 * 1024 + 512]
                    a_i = Abig[:, k2 * 1024 + 512: (k2 + 1) * 1024]
                    osl = slice(h * 512, (h + 1) * 512)
                    nc.tensor.matmul(Xr2[:, osl], ct["cw3r"], a_r, start=True, stop=False)
                    nc.tensor.matmul(Xr2[:, osl], ct["cw3n"], a_i, start=False, stop=True)
                    nc.tensor.matmul(Xi2[:, osl], ct["cw3i"], a_r, start=True, stop=False)
                    nc.tensor.matmul(Xi2[:, osl], ct["cw3r"], a_i, start=False, stop=True)
                zsl = slice(q * 1024, (q + 1) * 1024)
                xi4 = tp.tile([128, 1024], bf16, tag="xi4", name="xi4")
                nc.scalar.activation(xi4[:], Xi2[:], AF.Copy, scale=4.0)       # 4Xi
                xr4 = tp.tile([128, 1024], bf16, tag="xr4", name="xr4")
                nc.scalar.activation(xr4[:], Xr2[:], AF.Copy, scale=4.0)       # 4Xr
                m2 = tp.tile([128, 1024], bf16, tag="m2", name="m2")
                nc.vector.tensor_mul(m2[:], xi4[:], xi4[:])                    # 16Xi^2
                # Zi16 = (2*4Xr) * (4Xi) = 32XrXi = 16*(2XrXi) = 16Zi
                nc.vector.scalar_tensor_tensor(Zi4[:, zsl], xr4[:], 2.0, xi4[:], AO.mult, AO.mult)
                m1 = tp.tile([128, 1024], bf16, tag="m1", name="m1")
                nc.vector.tensor_mul(m1[:], xr4[:], xr4[:])                    # 16Xr^2
                nc.gpsimd.tensor_sub(Zr4[:, zsl], m1[:], m2[:])                # 16Zr

            def i1_pair(q):
                Ur2 = psa.tile([128, 1024], f32, tag="pA", name="Ur2")
                Ui2 = psa.tile([128, 1024], f32, tag="pB", name="Ui2")
                for h in range(2):
                    k2 = 2 * q + h
                    gsl = slice(k2 * 128, (k2 + 1) * 128)
                    zs = slice(q * 1024 + h * 512, q * 1024 + (h + 1) * 512)
                    osl = slice(h * 512, (h + 1) * 512)
                    nc.tensor.matmul(Ur2[:, osl], Gqr[:, gsl], Zr4[:, zs], start=True, stop=False)
                    nc.tensor.matmul(Ur2[:, osl], Gqni[:, gsl], Zi4[:, zs], start=False, stop=True)
                    nc.tensor.matmul(Ui2[:, osl], Gqi[:, gsl], Zr4[:, zs], start=True, stop=False)
                    nc.tensor.matmul(Ui2[:, osl], Gqr[:, gsl], Zi4[:, zs], start=False, stop=True)
                u_out_r = bass.AP(Ubig.tensor, Ubig[:].offset + (2 * q) * 1024,
                                  [[PITCH_U, 128], [1024, 2], [1, 512]])
                u_out_i = bass.AP(Ubig.tensor, Ubig[:].offset + (2 * q) * 1024 + 512,
                                  [[PITCH_U, 128], [1024, 2], [1, 512]])
                nc.scalar.activation(u_out_r, Ur2[:], AF.Copy)
                nc.vector.tensor_copy(u_out_i, Ui2[:])

            for q in range(9):
                f3_pair(q)
                if q > 0:
                    i1_pair(q - 1)
            i1_pair(8)
            if DEBUG_DUMP:
                nc.sync.dma_start(out=dbg["dAbig"][:, :], in_=Abig[:])
                nc.sync.dma_start(out=dbg["dxt"][:, :], in_=xt_r[:])

            # ---- pivot-D (DVE queue, prefetched) + I2, 9 pairs + 1 single ----
            def pivot_d(b):
                cnt = FCNT[b]
                u2 = sp.tile([126, 1024], bf16, tag="u2", name="u2", bufs=4)
                in_ap = bass.AP(Ubig.tensor, Ubig[:].offset + (7 * b) * PITCH_U,
                                [[PITCH_U, cnt], [1, N2 * 1024]])
                out_ap = bass.AP(u2.tensor, u2[:].offset,
                                 [[1024, cnt * 18], [1, 1024]])
                nc.gpsimd.dma_start(out=out_ap, in_=in_ap)
                return u2

            sreg = bp.tile([112, 8192], bf16, tag="big3", name="sreg")
            u2q = [pivot_d(0), pivot_d(1)]
            for p in range(10):
                blocks = [2 * p] if p == 9 else [2 * p, 2 * p + 1]
                u2s = [u2q.pop(0) for _ in blocks]
                for b in range(2 * p + 2, min(2 * p + 4, NB)):
                    u2q.append(pivot_d(b))
                yr2 = psa.tile([128, 1024], f32, tag="pA", name="yr2")
                yi2 = psa.tile([128, 1024], f32, tag="pB", name="yi2")
                for h, b in enumerate(blocks):
                    K = 18 * FCNT[b]
                    M = 16 * FCNT[b]
                    off = b * 112
                    osl = slice(h * 512, (h + 1) * 512)
                    u2 = u2s[h]
                    nc.tensor.matmul(yr2[:M, osl], ct["ci2r"][:K, off:off + M], u2[:K, 0:512], start=True, stop=False)
                    nc.tensor.matmul(yr2[:M, osl], ct["ci2n"][:K, off:off + M], u2[:K, 512:1024], start=False, stop=True)
                    nc.tensor.matmul(yi2[:M, osl], ct["ci2i"][:K, off:off + M], u2[:K, 0:512], start=True, stop=False)
                    nc.tensor.matmul(yi2[:M, osl], ct["ci2r"][:K, off:off + M], u2[:K, 512:1024], start=False, stop=True)
                Mp = 32 if p == 9 else 112
                W = 512 if p == 9 else 1024
                sb = (p % 2) * 4096
                s1 = sreg[:, sb + 0: sb + 1024]
                s2 = sreg[:, sb + 1024: sb + 2048]
                ss = sreg[:, sb + 2048: sb + 3072]
                yis = sreg[:, sb + 3072: sb + 4096]
                nc.scalar.activation(s1[:Mp, :W], yr2[:Mp, :W], AF.Square)
                nc.vector.tensor_copy(yis[:Mp, :W], yi2[:Mp, :W])
                nc.vector.tensor_mul(s2[:Mp, :W], yis[:Mp, :W], yis[:Mp, :W])
                nc.vector.tensor_add(ss[:Mp, :W], s1[:Mp, :W], s2[:Mp, :W])
                ya = sp.tile([112, 1024], bf16, tag="ya", name="ya", bufs=2)
                nc.scalar.activation(ya[:Mp, :W], ss[:Mp, :W], AF.Sqrt)
                if p == 9:
                    nc.sync.dma_start(out=yraw[18 * 112: 18 * 112 + 32, :], in_=ya[:32, :512])
                else:
                    nc.sync.dma_start(out=yraw[2 * p * 112: (2 * p + 1) * 112, :], in_=ya[:112, 0:512])
                    nc.sync.dma_start(out=yraw[(2 * p + 1) * 112: (2 * p + 2) * 112, :], in_=ya[:112, 512:1024])

            if DEBUG_DUMP:
                nc.sync.dma_start(out=dbg["dZr"][:, :], in_=Zr4[:])
                nc.sync.dma_start(out=dbg["dZi"][:, :], in_=Zi4[:])
                nc.sync.dma_start(out=dbg["dU"][:, :], in_=Ubig[:])

    nc.compile()
    return nc


_NC_CACHE = None


def _prep_inputs(x_real, x_imag):
    """Pure data movement: permute+transpose x into the [1280, B] device layout."""
    valid = XRMAP >= 0
    out = []
    for x in (x_real, x_imag):
        xt = np.zeros((XROWS, B), ml_dtypes.bfloat16)
        xt[valid] = np.ascontiguousarray(x[:, XRMAP[valid]].T)
        out.append(xt)
    return out


def kernel(**inputs):
    global _NC_CACHE
    x_real = np.ascontiguousarray(inputs["x_real"], dtype=np.float32)
    x_imag = np.ascontiguousarray(inputs["x_imag"], dtype=np.float32)
    w0_real = np.ascontiguousarray(inputs["w0_real"], dtype=np.float32)
    w0_imag = np.ascontiguousarray(inputs["w0_imag"], dtype=np.float32)
    wl_real = np.ascontiguousarray(inputs["wl_real"], dtype=np.float32)
    wl_imag = np.ascontiguousarray(inputs["wl_imag"], dtype=np.float32)

    xtr_full, xti_full = _prep_inputs(x_real, x_imag)

    wpack = np.stack([w0_real[0:128], w0_imag[0:128],
                      wl_real[0:128], wl_imag[0:128],
                      wl_real[128:256], wl_imag[128:256]], axis=1).astype(np.float32)
    wtail = np.array([[w0_real[128], w0_imag[128], wl_real[256], wl_imag[256]]],
                     np.float32)
    const_maps = {nm: np.ascontiguousarray(arr) for nm, arr in CONSTS.items()}
    in_maps = []
    for cid in range(NCORES):
        cols = slice(cid * BCORE, (cid + 1) * BCORE)
        m = {
            "xtr": np.ascontiguousarray(xtr_full[:, cols]),
            "xti": np.ascontiguousarray(xti_full[:, cols]),
            "wpack": wpack, "wtail": wtail,
        }
        m.update(const_maps)
        in_maps.append(m)

    if _NC_CACHE is None:
        _NC_CACHE = build_nc()
    res = run_bass_kernel_spmd(_NC_CACHE, in_maps, core_ids=list(range(NCORES)))

    out = np.empty((B, CLASS_NUM), np.float32)
    for cid in range(NCORES):
        yr = np.asarray(res.results[cid]["yraw"]).astype(np.float32)  # [2048, 512]
        out[cid * BCORE:(cid + 1) * BCORE, :] = yr[YINV, :].T
    return out


# revision 29
# speedup vs baseline: 1.8401x; 1.0729x over previous
"""Trainium2 Bass kernel for nn_CNN_Comp_29240137351522 (dense_cnn).

Math:  y = |IFFT_N( FFT_N(x)^2 * C )[255:2303]|,  C = FFT_N(w0)^2 * FFT_N(wl) / N
with N = 2304 = 128*18.  2304 >= 2303-ish: circular aliasing only contaminates
samples n < 255, which the center crop [255, 2303) never reads, so the chained
full convolutions are exact on the cropped output.

Device decomposition (per core, data-parallel over batch, b = 512 samples):
  N = N1*N2, N1=128, N2=18;  time n = n2*128+n1,  freq k = k1*18+k2
  F1 (contract n2<8, PE, 19 blocks of <=7 n1-values, twiddle folded)
  F3 (contract n1, PE, shared W128 DFT, bf16)      -> X[k1, (k2,b)]
  square (ACT/DVE/Pool fused eviction)             -> Zr4 = 4(Xr^2-Xi^2), Zi2 = 4XrXi
  I1 (contract k1, PE, per-k2 G = (C/4N)-row-scaled inverse DFT), bf16
  I2 (contract k2, PE, 19 blocks of <=7 n1'-values, twiddle folded,
      exactly 16 valid n2' outputs per n1' -> 2048 rows), bf16
  |.| fused into I2 eviction; raw bf16 tiles stored to DRAM, unscrambled on host.

Host does data movement only: batch shard, pre-transposed/permuted copy of x
(so no on-device transposes are needed), and the inverse row->output-column
unscramble of the raw result.
"""

import numpy as np
import ml_dtypes

import concourse.bass as bass
import concourse.bacc as bacc
import concourse.mybir as mybir
from concourse.tile import TileContext
from concourse.bass_utils import run_bass_kernel_spmd

# ---------------- static problem config ----------------
B, NX = 4096, 1024
K0, KL = 129, 257
N = 2304
N1, N2 = 128, 18
NCORES = 8
BCORE = B // NCORES          # 512
FCNT = [7] * 18 + [2]        # il-count per block; n1 = 7*b + il
NB = 19
CROP0 = 255
CLASS_NUM = 2048
XROWS = 1280                 # 10 tiles of 128 rows, 2 blocks per tile
PITCH_A = N2 * 1024          # Abig free size  (k2, plane, b)
PITCH_U = N2 * 1024          # Ubig free size  (k2, plane, b)

f32 = mybir.dt.float32
f32r = mybir.dt.float32r
bf16 = mybir.dt.bfloat16
AO = mybir.AluOpType
AF = mybir.ActivationFunctionType


def _w(num, den):
    return np.exp(-2j * np.pi * np.asarray(num, np.float64) / den)


# ---------------- host-side constant arrays ----------------
def _build_consts():
    c = {}
    k1g = np.arange(N1)
    k2g = np.arange(N2)
    nh = np.arange(128)

    # F1 lhsT: [112, 10*126]; pair slot s=b%2 at rows [56s,56s+56);
    # block b at cols [(b//2)*126, ...); rows (il)*8+n2, cols il*18+k2;
    # value W18[n2,k2] * W2304^{n1 k2}, n1 = 7b+il
    f1 = np.zeros((120, 10 * 126), np.complex128)
    n2g8 = np.arange(8)
    for b in range(NB):
        s, t = b % 2, b // 2
        for il in range(FCNT[b]):
            n1 = 7 * b + il
            blk = _w(np.outer(n2g8, k2g), N2) * _w(n1 * k2g, N)[None, :]
            f1[64 * s + il * 8: 64 * s + il * 8 + 8,
               t * 126 + il * 18: t * 126 + (il + 1) * 18] = blk
    c["cf1r"] = f1.real.astype(ml_dtypes.bfloat16)
    c["cf1i"] = f1.imag.astype(ml_dtypes.bfloat16)
    c["cf1n"] = (-f1.imag).astype(ml_dtypes.bfloat16)

    # F3 lhsT (shared, bf16): W128[n1,k1]
    w3 = _w(np.outer(nh, k1g), N1)
    c["cw3r"] = w3.real.astype(ml_dtypes.bfloat16)
    c["cw3i"] = w3.imag.astype(ml_dtypes.bfloat16)
    c["cw3n"] = (-w3.imag).astype(ml_dtypes.bfloat16)
    # f32 copies for the weight DFT (precision)
    c["cw3rf"] = w3.real.astype(np.float32)
    c["cw3if"] = w3.imag.astype(np.float32)
    c["cw3nf"] = (-w3.imag).astype(np.float32)

    # I1 base: W128i[k1,n1] (f32, G built on device)
    wi = _w(-np.outer(k1g, np.arange(N1)), N1)
    c["cwir"] = wi.real.astype(ml_dtypes.bfloat16)
    c["cwii"] = wi.imag.astype(ml_dtypes.bfloat16)

    # I2 lhsT: [126, 2048]; block b cols [b*112, b*112+16*cnt);
    # rows il*18+k2, cols il*16+j; n2'(j) = (1 if n1'==127 else 2)+j
    i2 = np.zeros((126, 2048), np.complex128)
    for b in range(NB):
        for il in range(FCNT[b]):
            n1p = 7 * b + il
            base = 1 if n1p == 127 else 2
            n2p = base + np.arange(16)
            blk = _w(-np.outer(k2g, n2p), N2) * _w(-n1p * k2g, N)[:, None]
            i2[il * 18: (il + 1) * 18,
               b * 112 + il * 16: b * 112 + (il + 1) * 16] = blk
    c["ci2r"] = i2.real.astype(ml_dtypes.bfloat16)
    c["ci2i"] = i2.imag.astype(ml_dtypes.bfloat16)
    c["ci2n"] = (-i2.imag).astype(ml_dtypes.bfloat16)

    # weight-DFT rhs constants
    t129 = _w(np.outer(nh, k2g), N)
    c["ct1r"] = t129.real.astype(np.float32)
    c["ct1i"] = t129.imag.astype(np.float32)
    te1 = _w(k2g, N2)
    t257b = t129 * te1[None, :]
    c["ct2r"] = t257b.real.astype(np.float32)
    c["ct2i"] = t257b.imag.astype(np.float32)
    c["te1r"] = te1.real.astype(np.float32).reshape(1, N2)
    c["te1i"] = te1.imag.astype(np.float32).reshape(1, N2)
    te2 = _w(k2g, 9)
    c["te2r"] = te2.real.astype(np.float32).reshape(1, N2)
    c["te2i"] = te2.imag.astype(np.float32).reshape(1, N2)

    c["ones1"] = np.ones((1, 128), np.float32)

    # ---- pack into few DRAM tensors (fewer DMAs; device slices views) ----
    p = {}
    p["wdftf"] = np.concatenate([c[k] for k in
        ["ct1r", "ct1i", "ct2r", "ct2i", "cw3rf", "cw3if", "cw3nf"]], axis=1)
    p["smalls"] = np.concatenate([c[k] for k in
        ["te1r", "te1i", "te2r", "te2i", "ones1"]], axis=1)
    p["cwaux"] = np.concatenate([c["cwir"], c["cwii"]], axis=1)
    p["cf1p"] = np.concatenate([c["cf1r"], c["cf1i"], c["cf1n"]], axis=1)
    p["cw3p"] = np.concatenate([c["cw3r"], c["cw3i"], c["cw3n"]], axis=1)
    p["ci2p"] = np.concatenate([c["ci2r"], c["ci2i"], c["ci2n"]], axis=1)
    return p


CONSTS = _build_consts()


def _xrow_map():
    """DRAM row r of xp_t -> original column n of x (or -1 for pad)."""
    rmap = np.full(XROWS, -1, np.int64)
    for r in range(XROWS):
        t, rr = divmod(r, 128)
        s, rrr = divmod(rr, 64)
        if rrr >= 56:
            continue
        il, n2 = divmod(rrr, 8)
        b = 2 * t + s
        if b >= NB or il >= FCNT[b]:
            continue
        n1 = 7 * b + il
        rmap[r] = n2 * 128 + n1
    return rmap


def _ycol_map():
    """yraw row r -> output column (n - 255)."""
    cols = np.empty(2048, np.int64)
    r = 0
    for b in range(NB):
        for il in range(FCNT[b]):
            n1p = 7 * b + il
            base = 1 if n1p == 127 else 2
            for j in range(16):
                cols[b * 112 + il * 16 + j] = (base + j) * 128 + n1p - CROP0
                r += 1
    return cols


XRMAP = _xrow_map()
YCOLS = _ycol_map()
YINV = np.empty(2048, np.int64)
YINV[YCOLS] = np.arange(2048)


# ---------------- bass kernel builder ----------------
DEBUG_DUMP = False


def build_nc():
    nc = bacc.Bacc("TRN2", target_bir_lowering=False, debug=False, num_devices=NCORES)

    d = {}
    d["xtr"] = nc.dram_tensor("xtr", [XROWS, BCORE], bf16, kind="ExternalInput")
    d["xti"] = nc.dram_tensor("xti", [XROWS, BCORE], bf16, kind="ExternalInput")
    d["wpack"] = nc.dram_tensor("wpack", [128, 6], f32, kind="ExternalInput")
    d["wtail"] = nc.dram_tensor("wtail", [1, 4], f32, kind="ExternalInput")
    cdt = {"cf1p": bf16, "cw3p": bf16, "ci2p": bf16, "cwaux": bf16}
    for nm, arr in CONSTS.items():
        d[nm] = nc.dram_tensor(nm, list(arr.shape), cdt.get(nm, f32), kind="ExternalInput")
    yraw = nc.dram_tensor("yraw", [2048, BCORE], bf16, kind="ExternalOutput")
    dbg = {}
    if DEBUG_DUMP:
        dbg["dAbig"] = nc.dram_tensor("dAbig", [128, N2 * 1024], bf16, kind="ExternalOutput")
        dbg["dZr"] = nc.dram_tensor("dZr", [128, N2 * 512], bf16, kind="ExternalOutput")
        dbg["dZi"] = nc.dram_tensor("dZi", [128, N2 * 512], bf16, kind="ExternalOutput")
        dbg["dU"] = nc.dram_tensor("dU", [128, N2 * 1024], bf16, kind="ExternalOutput")
        dbg["dxt"] = nc.dram_tensor("dxt", [128, 10 * BCORE], bf16, kind="ExternalOutput")

    with TileContext(nc) as tc:
        with (
            tc.tile_pool(name="cp", bufs=1) as cp,          # consts + persistent
            tc.tile_pool(name="bp", bufs=1) as bp,          # big tiles (tag-aliased)
            tc.tile_pool(name="sp", bufs=2) as sp,          # rotating bf16 tiles
            tc.tile_pool(name="tp", bufs=2) as tp,          # rotating f32 tmps
            tc.tile_pool(name="psa", bufs=2, space="PSUM") as psa,  # 2 tags x 2 bufs x 2 banks
        ):
            # ---- load packed weights/consts (few DMAs; critical-path first) ----
            wpk = cp.tile([128, 6], f32, tag="wpk", name="wpk")
            nc.sync.dma_start(out=wpk[:], in_=d["wpack"][:, :])
            wtl = cp.tile([1, 4], f32, tag="wtl", name="wtl")
            nc.sync.dma_start(out=wtl[:], in_=d["wtail"][:, :])
            wc = {"w0r_c": wpk[:, 0:1], "w0i_c": wpk[:, 1:2],
                  "wlr_c1": wpk[:, 2:3], "wli_c1": wpk[:, 3:4],
                  "wlr_c2": wpk[:, 4:5], "wli_c2": wpk[:, 5:6],
                  "w0r_e": wtl[:, 0:1], "w0i_e": wtl[:, 1:2],
                  "wlr_e": wtl[:, 2:3], "wli_e": wtl[:, 3:4]}

            pk = {}
            for nm, eng in [("wdftf", nc.sync), ("smalls", nc.sync), ("cf1p", nc.sync),
                            ("cwaux", nc.scalar), ("cw3p", nc.scalar), ("ci2p", nc.scalar)]:
                arr = CONSTS[nm]
                t = cp.tile(list(arr.shape), cdt.get(nm, f32), tag=nm, name=nm)
                eng.dma_start(out=t[:], in_=d[nm][:, :])
                pk[nm] = t
            ct = {
                "ct1r": pk["wdftf"][:, 0:18], "ct1i": pk["wdftf"][:, 18:36],
                "ct2r": pk["wdftf"][:, 36:54], "ct2i": pk["wdftf"][:, 54:72],
                "cw3rf": pk["wdftf"][:, 72:200], "cw3if": pk["wdftf"][:, 200:328],
                "cw3nf": pk["wdftf"][:, 328:456],
                "te1r": pk["smalls"][:1, 0:18], "te1i": pk["smalls"][:1, 18:36],
                "te2r": pk["smalls"][:1, 36:54], "te2i": pk["smalls"][:1, 54:72],
                "ones1": pk["smalls"][:1, 72:200],
                "cwir": pk["cwaux"][:, 0:128], "cwii": pk["cwaux"][:, 128:256],
                "cf1r": pk["cf1p"][:, 0:1260], "cf1i": pk["cf1p"][:, 1260:2520],
                "cf1n": pk["cf1p"][:, 2520:3780],
                "cw3r": pk["cw3p"][:, 0:128], "cw3i": pk["cw3p"][:, 128:256],
                "cw3n": pk["cw3p"][:, 256:384],
                "ci2r": pk["ci2p"][:, 0:2048], "ci2i": pk["ci2p"][:, 2048:4096],
                "ci2n": pk["ci2p"][:, 4096:6144],
            }

            # ---- load x planes in 4 chunks per plane (F1 starts early) ----
            xt_r = bp.tile([128, 10 * BCORE], bf16, tag="big1", name="xt_r")
            xt_i = bp.tile([128, 10 * BCORE], bf16, tag="big2", name="xt_i")
            for c0, ntile in ((0, 3), (3, 3), (6, 3), (9, 1)):
                for xt, srcn in ((xt_r, "xtr"), (xt_i, "xti")):
                    nc.sync.dma_start(
                        out=xt[:, 512 * c0: 512 * (c0 + ntile)],
                        in_=bass.AP(d[srcn][:, :].tensor, 128 * c0 * BCORE,
                                    [[BCORE, 128], [128 * BCORE, ntile], [1, BCORE]]),
                    )

            # ---- weight DFT: W0, WL [128, 18] ----
            def build_rhs(tr, ti, cr_, ci_, out_r, out_i):
                tmp = tp.tile([tr.shape[0], N2], f32, tag="wtmp", name="wtmp")
                nc.vector.tensor_scalar(tmp[:], ti, ci_, None, AO.mult)
                nc.vector.scalar_tensor_tensor(out_r[:], tr, cr_, tmp[:], AO.mult, AO.subtract)
                tmp2 = tp.tile([tr.shape[0], N2], f32, tag="wtmp2", name="wtmp2")
                nc.vector.tensor_scalar(tmp2[:], tr, ci_, None, AO.mult)
                nc.vector.scalar_tensor_tensor(out_i[:], ti, cr_, tmp2[:], AO.mult, AO.add)

            def weight_dft(chunks, tail, ps, c0):
                ps_r = ps[:, c0: c0 + N2]
                ps_i = ps[:, c0 + N2: c0 + 2 * N2]
                rhs = []
                for (t_r, t_i, colr, coli) in chunks:
                    rr = sp.tile([128, N2], f32, tag="wrhs_r", name="wrhs_r")
                    ri = sp.tile([128, N2], f32, tag="wrhs_i", name="wrhs_i")
                    build_rhs(t_r, t_i, colr, coli, rr, ri)
                    rhs.append((rr, ri))
                te_r, te_i, er, ei = tail
                tr_ = sp.tile([1, N2], f32, tag="wtail_r", name="wtail_r")
                ti_ = sp.tile([1, N2], f32, tag="wtail_i", name="wtail_i")
                tmp = tp.tile([1, N2], f32, tag="wtmp3", name="wtmp3")
                nc.vector.tensor_scalar(tmp[:], te_i, ei, None, AO.mult)
                nc.vector.scalar_tensor_tensor(tr_[:], te_r, er, tmp[:], AO.mult, AO.subtract)
                tmp2 = tp.tile([1, N2], f32, tag="wtmp4", name="wtmp4")
                nc.vector.tensor_scalar(tmp2[:], te_r, ei, None, AO.mult)
                nc.vector.scalar_tensor_tensor(ti_[:], te_i, er, tmp2[:], AO.mult, AO.add)
                first = True
                for (rr, ri) in rhs:
                    nc.tensor.matmul(ps_r, ct["cw3rf"], rr[:], start=first, stop=False)
                    nc.tensor.matmul(ps_r, ct["cw3nf"], ri[:], start=False, stop=False)
                    first = False
                nc.tensor.matmul(ps_r, ct["ones1"], tr_[:], start=False, stop=True)
                first = True
                for (rr, ri) in rhs:
                    nc.tensor.matmul(ps_i, ct["cw3if"], rr[:], start=first, stop=False)
                    nc.tensor.matmul(ps_i, ct["cw3rf"], ri[:], start=False, stop=False)
                    first = False
                nc.tensor.matmul(ps_i, ct["ones1"], ti_[:], start=False, stop=True)

            ps_w = psa.tile([128, 1024], f32, tag="pB", name="ps_w")
            weight_dft(
                [(ct["ct1r"], ct["ct1i"], wc["w0r_c"], wc["w0i_c"])],
                (ct["te1r"], ct["te1i"], wc["w0r_e"], wc["w0i_e"]),
                ps_w, 0,
            )
            weight_dft(
                [(ct["ct1r"], ct["ct1i"], wc["wlr_c1"], wc["wli_c1"]),
                 (ct["ct2r"], ct["ct2i"], wc["wlr_c2"], wc["wli_c2"])],
                (ct["te2r"], ct["te2i"], wc["wlr_e"], wc["wli_e"]),
                ps_w, 2 * N2,
            )
            Wall = cp.tile([128, 4 * N2], f32, tag="Wall", name="Wall")
            nc.vector.tensor_copy(Wall[:], ps_w[:, 0: 4 * N2])
            W0r = Wall[:, 0:N2]
            W0i = Wall[:, N2: 2 * N2]
            WLr = Wall[:, 2 * N2: 3 * N2]
            WLi = Wall[:, 3 * N2: 4 * N2]

            # ---- Cq = W0^2 * WL / (16N)  [128, 18] ----
            # Device evicts Zr16 = 16Zr and Zi16 = 16Zi (same scale), so all
            # three G variants are built from Cq = C/(16N).
            Cqr = cp.tile([128, N2], f32, tag="Cqr", name="Cqr")
            Cqi = cp.tile([128, N2], f32, tag="Cqi", name="Cqi")
            nCqr = cp.tile([128, N2], f32, tag="nCqr", name="nCqr")
            ta = tp.tile([128, N2], f32, tag="ca", name="ca")
            tb = tp.tile([128, N2], f32, tag="cb", name="cb")
            tm1 = tp.tile([128, N2], f32, tag="cm1", name="cm1")
            tm2 = tp.tile([128, N2], f32, tag="cm2", name="cm2")
            nc.vector.tensor_mul(tm1[:], W0r, W0r)
            nc.vector.tensor_mul(tm2[:], W0i, W0i)
            nc.vector.tensor_sub(ta[:], tm1[:], tm2[:])
            nc.vector.tensor_mul(tm1[:], W0r, W0i)
            nc.vector.tensor_add(tb[:], tm1[:], tm1[:])
            nc.vector.tensor_mul(tm1[:], ta[:], WLr)
            nc.vector.tensor_mul(tm2[:], tb[:], WLi)
            nc.vector.tensor_sub(tm1[:], tm1[:], tm2[:])
            nc.scalar.mul(Cqr[:], tm1[:], 1.0 / (16 * N))
            nc.scalar.mul(nCqr[:], tm1[:], -1.0 / (16 * N))
            nc.vector.tensor_mul(tm1[:], ta[:], WLi)
            nc.vector.tensor_mul(tm2[:], tb[:], WLr)
            nc.vector.tensor_add(tm1[:], tm1[:], tm2[:])
            nc.scalar.mul(Cqi[:], tm1[:], 1.0 / (16 * N))

            # ---- G variants (bf16): with inputs (Zr4, Zi4) = (4Zr, 4Zi):
            # Ur = Gqr.Zr4 + Gqni.Zi4 ; Ui = Gqi.Zr4 + Gqr.Zi4
            Gqr = cp.tile([128, N2 * 128], bf16, tag="Gqr", name="Gqr")
            Gqi = cp.tile([128, N2 * 128], bf16, tag="Gqi", name="Gqi")
            Gqni = cp.tile([128, N2 * 128], bf16, tag="Gqni", name="Gqni")

            def g_build(k2):
                sl = slice(k2 * 128, (k2 + 1) * 128)
                cqr_ = Cqr[:, k2:k2 + 1]
                cqi_ = Cqi[:, k2:k2 + 1]
                g1 = tp.tile([128, 128], bf16, tag="g1", name="g1", bufs=2)
                nc.scalar.mul(g1[:], ct["cwii"], cqi_)
                nc.vector.scalar_tensor_tensor(Gqr[:, sl], ct["cwir"], cqr_, g1[:], AO.mult, AO.subtract)
                g2 = tp.tile([128, 128], bf16, tag="g2", name="g2", bufs=2)
                nc.scalar.mul(g2[:], ct["cwir"], cqi_)
                nc.vector.scalar_tensor_tensor(Gqi[:, sl], ct["cwii"], cqr_, g2[:], AO.mult, AO.add)
                # Gqni = -(cwii*Cqr + cwir*Cqi) = (cwii * nCqr) - g2
                nc.vector.scalar_tensor_tensor(Gqni[:, sl], ct["cwii"], nCqr[:, k2:k2 + 1], g2[:], AO.mult, AO.subtract)

            # ---- F1: 9 pairs + 1 single ----
            Abig = bp.tile([128, PITCH_A], bf16, tag="big3", name="Abig")

            def f1_block(pr, pi, h, b):
                s, t = b % 2, b // 2
                K = 8 * FCNT[b]
                M = 18 * FCNT[b]
                rr = xt_r[64 * s: 64 * s + K, 512 * t: 512 * (t + 1)]
                ri = xt_i[64 * s: 64 * s + K, 512 * t: 512 * (t + 1)]
                lr = ct["cf1r"][64 * s: 64 * s + K, t * 126: t * 126 + M]
                li = ct["cf1i"][64 * s: 64 * s + K, t * 126: t * 126 + M]
                ln = ct["cf1n"][64 * s: 64 * s + K, t * 126: t * 126 + M]
                osl = slice(h * 512, (h + 1) * 512)
                nc.tensor.matmul(pr[:M, osl], lr, rr, start=True, stop=False)
                nc.tensor.matmul(pr[:M, osl], ln, ri, start=False, stop=True)
                nc.tensor.matmul(pi[:M, osl], li, rr, start=True, stop=False)
                nc.tensor.matmul(pi[:M, osl], lr, ri, start=False, stop=True)

            def f1_pivot(ag, h, b, npart):
                # ag partition = il*18+k2 ; cols (plane, blkhalf, b)
                cnt = FCNT[b]
                in_ap = bass.AP(ag.tensor, ag[:].offset + h * 512,
                                [[2048, cnt * 18], [1024, 2], [1, 512]])
                out_ap = bass.AP(Abig.tensor, Abig[:].offset + (7 * b) * PITCH_A,
                                 [[PITCH_A, cnt], [1024, 18], [512, 2], [1, 512]])
                nc.sync.dma_start(out=out_ap, in_=in_ap)

            for p in range(10):
                blocks = [2 * p] if p == 9 else [2 * p, 2 * p + 1]
                pr2 = psa.tile([128, 1024], f32, tag="pA", name="pr2")
                pi2 = psa.tile([128, 1024], f32, tag="pB", name="pi2")
                for h, b in enumerate(blocks):
                    f1_block(pr2, pi2, h, b)
                if p == 9:
                    ag = sp.tile([36, 2048], bf16, tag="ag", name="ag")
                    nc.scalar.activation(ag[:36, 0:512], pr2[:36, 0:512], AF.Copy)
                    nc.vector.tensor_copy(ag[:36, 1024:1536], pi2[:36, 0:512])
                    f1_pivot(ag, 0, 18, 36)
                else:
                    ag = sp.tile([126, 2048], bf16, tag="ag", name="ag")
                    nc.scalar.activation(ag[:, 0:1024], pr2[:126, :], AF.Copy)
                    nc.vector.tensor_copy(ag[:, 1024:2048], pi2[:126, :])
                    f1_pivot(ag, 0, 2 * p, 126)
                    f1_pivot(ag, 1, 2 * p + 1, 126)
                if 2 * p + 1 < N2:
                    g_build(2 * p)
                    g_build(2 * p + 1)

            # ---- F3 + square + I1 (I1 runs one pair behind F3), 9 k2-pairs ----
            Zr4 = bp.tile([128, N2 * 512], bf16, tag="big2", name="Zr4")   # aliases xt_i
            Zi4 = bp.tile([128, N2 * 512], bf16, tag="big4", name="Zi4")
            Ubig = bp.tile([128, PITCH_U], bf16, tag="big1", name="Ubig")  # aliases xt_r

            def f3_pair(q):
                Xr2 = psa.tile([128, 1024], f32, tag="pA", name="Xr2")
                Xi2 = psa.tile([128, 1024], f32, tag="pB", name="Xi2")
                for h in range(2):
                    k2 = 2 * q + h
                    a_r = Abig[:, k2 * 1024: k2 * 1024 + 512]
                    a_i = Abig[:, k2 * 1024 + 512: (k2 + 1) * 1024]
                    osl = slice(h * 512, (h + 1) * 512)
                    nc.tensor.matmul(Xr2[:, osl], ct["cw3r"], a_r, start=True, stop=False)
                    nc.tensor.matmul(Xr2[:, osl], ct["cw3n"], a_i, start=False, stop=True)
                    nc.tensor.matmul(Xi2[:, osl], ct["cw3i"], a_r, start=True, stop=False)
                    nc.tensor.matmul(Xi2[:, osl], ct["cw3r"], a_i, start=False, stop=True)
                zsl = slice(q * 1024, (q + 1) * 1024)
                xi4 = tp.tile([128, 1024], bf16, tag="xi4", name="xi4")
                nc.scalar.activation(xi4[:], Xi2[:], AF.Copy, scale=4.0)       # 4Xi
                xr4 = tp.tile([128, 1024], bf16, tag="xr4", name="xr4")
                nc.scalar.activation(xr4[:], Xr2[:], AF.Copy, scale=4.0)       # 4Xr
                m2 = tp.tile([128, 1024], bf16, tag="m2", name="m2")
                nc.vector.tensor_mul(m2[:], xi4[:], xi4[:])                    # 16Xi^2
                # Zi16 = (2*4Xr) * (4Xi) = 32XrXi = 16*(2XrXi) = 16Zi
                nc.vector.scalar_tensor_tensor(Zi4[:, zsl], xr4[:], 2.0, xi4[:], AO.mult, AO.mult)
                m1 = tp.tile([128, 1024], bf16, tag="m1", name="m1")
                nc.vector.tensor_mul(m1[:], xr4[:], xr4[:])                    # 16Xr^2
                nc.gpsimd.tensor_sub(Zr4[:, zsl], m1[:], m2[:])                # 16Zr

            def i1_pair(q):
                Ur2 = psa.tile([128, 1024], f32, tag="pA", name="Ur2")
                Ui2 = psa.tile([128, 1024], f32, tag="pB", name="Ui2")
                for h in range(2):
                    k2 = 2 * q + h
                    gsl = slice(k2 * 128, (k2 + 1) * 128)
                    zs = slice(q * 1024 + h * 512, q * 1024 + (h + 1) * 512)
                    osl = slice(h * 512, (h + 1) * 512)
                    nc.tensor.matmul(Ur2[:, osl], Gqr[:, gsl], Zr4[:, zs], start=True, stop=False)
                    nc.tensor.matmul(Ur2[:, osl], Gqni[:, gsl], Zi4[:, zs], start=False, stop=True)
                    nc.tensor.matmul(Ui2[:, osl], Gqi[:, gsl], Zr4[:, zs], start=True, stop=False)
                    nc.tensor.matmul(Ui2[:, osl], Gqr[:, gsl], Zi4[:, zs], start=False, stop=True)
                u_out_r = bass.AP(Ubig.tensor, Ubig[:].offset + (2 * q) * 1024,
                                  [[PITCH_U, 128], [1024, 2], [1, 512]])
                u_out_i = bass.AP(Ubig.tensor, Ubig[:].offset + (2 * q) * 1024 + 512,
                                  [[PITCH_U, 128], [1024, 2], [1, 512]])
                nc.scalar.activation(u_out_r, Ur2[:], AF.Copy)
                nc.vector.tensor_copy(u_out_i, Ui2[:])

            for q in range(9):
                f3_pair(q)
                if q > 1:
                    i1_pair(q - 2)
            i1_pair(7)
            i1_pair(8)
            if DEBUG_DUMP:
                nc.sync.dma_start(out=dbg["dAbig"][:, :], in_=Abig[:])
                nc.sync.dma_start(out=dbg["dxt"][:, :], in_=xt_r[:])

            # ---- pivot-D (DVE queue, prefetched) + I2, 9 pairs + 1 single ----
            def pivot_d(b):
                cnt = FCNT[b]
                u2 = sp.tile([126, 1024], bf16, tag="u2", name="u2", bufs=4)
                in_ap = bass.AP(Ubig.tensor, Ubig[:].offset + (7 * b) * PITCH_U,
                                [[PITCH_U, cnt], [1, N2 * 1024]])
                out_ap = bass.AP(u2.tensor, u2[:].offset,
                                 [[1024, cnt * 18], [1, 1024]])
                nc.gpsimd.dma_start(out=out_ap, in_=in_ap)
                return u2

            sreg = bp.tile([112, 8192], bf16, tag="big3", name="sreg")
            u2q = [pivot_d(0), pivot_d(1)]
            for p in range(10):
                blocks = [2 * p] if p == 9 else [2 * p, 2 * p + 1]
                u2s = [u2q.pop(0) for _ in blocks]
                for b in range(2 * p + 2, min(2 * p + 4, NB)):
                    u2q.append(pivot_d(b))
                yr2 = psa.tile([128, 1024], f32, tag="pA", name="yr2")
                yi2 = psa.tile([128, 1024], f32, tag="pB", name="yi2")
                for h, b in enumerate(blocks):
                    K = 18 * FCNT[b]
                    M = 16 * FCNT[b]
                    off = b * 112
                    osl = slice(h * 512, (h + 1) * 512)
                    u2 = u2s[h]
                    nc.tensor.matmul(yr2[:M, osl], ct["ci2r"][:K, off:off + M], u2[:K, 0:512], start=True, stop=False)
                    nc.tensor.matmul(yr2[:M, osl], ct["ci2n"][:K, off:off + M], u2[:K, 512:1024], start=False, stop=True)
                    nc.tensor.matmul(yi2[:M, osl], ct["ci2i"][:K, off:off + M], u2[:K, 0:512], start=True, stop=False)
                    nc.tensor.matmul(yi2[:M, osl], ct["ci2r"][:K, off:off + M], u2[:K, 512:1024], start=False, stop=True)
                Mp = 32 if p == 9 else 112
                W = 512 if p == 9 else 1024
                sb = (p % 2) * 4096
                s1 = sreg[:, sb + 0: sb + 1024]
                s2 = sreg[:, sb + 1024: sb + 2048]
                ss = sreg[:, sb + 2048: sb + 3072]
                yis = sreg[:, sb + 3072: sb + 4096]
                nc.scalar.activation(s1[:Mp, :W], yr2[:Mp, :W], AF.Square)
                nc.vector.tensor_copy(yis[:Mp, :W], yi2[:Mp, :W])
                nc.vector.tensor_mul(s2[:Mp, :W], yis[:Mp, :W], yis[:Mp, :W])
                nc.vector.tensor_add(ss[:Mp, :W], s1[:Mp, :W], s2[:Mp, :W])
                ya = sp.tile([112, 1024], bf16, tag="ya", name="ya", bufs=2)
                nc.scalar.activation(ya[:Mp, :W], ss[:Mp, :W], AF.Sqrt)
                st = nc.sync if p % 2 == 0 else nc.scalar
                if p == 9:
                    st.dma_start(out=yraw[18 * 112: 18 * 112 + 32, :], in_=ya[:32, :512])
                else:
                    st.dma_start(out=yraw[2 * p * 112: (2 * p + 1) * 112, :], in_=ya[:112, 0:512])
                    st.dma_start(out=yraw[(2 * p + 1) * 112: (2 * p + 2) * 112, :], in_=ya[:112, 512:1024])

            if DEBUG_DUMP:
                nc.sync.dma_start(out=dbg["dZr"][:, :], in_=Zr4[:])
                nc.sync.dma_start(out=dbg["dZi"][:, :], in_=Zi4[:])
                nc.sync.dma_start(out=dbg["dU"][:, :], in_=Ubig[:])

    nc.compile()
    return nc


_NC_CACHE = None


def _prep_inputs(x_real, x_imag):
    """Pure data movement: permute+transpose x into the [1280, B] device layout."""
    valid = XRMAP >= 0
    out = []
    for x in (x_real, x_imag):
        xt = np.zeros((XROWS, B), ml_dtypes.bfloat16)
        xt[valid] = np.ascontiguousarray(x[:, XRMAP[valid]].T)
        out.append(xt)
    return out


def kernel(**inputs):
    global _NC_CACHE
    x_real = np.ascontiguousarray(inputs["x_real"], dtype=np.float32)
    x_imag = np.ascontiguousarray(inputs["x_imag"], dtype=np.float32)
    w0_real = np.ascontiguousarray(inputs["w0_real"], dtype=np.float32)
    w0_imag = np.ascontiguousarray(inputs["w0_imag"], dtype=np.float32)
    wl_real = np.ascontiguousarray(inputs["wl_real"], dtype=np.float32)
    wl_imag = np.ascontiguousarray(inputs["wl_imag"], dtype=np.float32)

    xtr_full, xti_full = _prep_inputs(x_real, x_imag)

    wpack = np.stack([w0_real[0:128], w0_imag[0:128],
                      wl_real[0:128], wl_imag[0:128],
                      wl_real[128:256], wl_imag[128:256]], axis=1).astype(np.float32)
    wtail = np.array([[w0_real[128], w0_imag[128], wl_real[256], wl_imag[256]]],
                     np.float32)
    const_maps = {nm: np.ascontiguousarray(arr) for nm, arr in CONSTS.items()}
    in_maps = []
    for cid in range(NCORES):
        cols = slice(cid * BCORE, (cid + 1) * BCORE)
        m = {
            "xtr": np.ascontiguousarray(xtr_full[:, cols]),
            "xti": np.ascontiguousarray(xti_full[:, cols]),
            "wpack": wpack, "wtail": wtail,
        }
        m.update(const_maps)
        in_maps.append(m)

    if _NC_CACHE is None:
        _NC_CACHE = build_nc()
    res = run_bass_kernel_spmd(_NC_CACHE, in_maps, core_ids=list(range(NCORES)))

    out = np.empty((B, CLASS_NUM), np.float32)
    for cid in range(NCORES):
        yr = np.asarray(res.results[cid]["yraw"]).astype(np.float32)  # [2048, 512]
        out[cid * BCORE:(cid + 1) * BCORE, :] = yr[YINV, :].T
    return out
